# revision 24
# baseline (speedup 1.0000x reference)
"""Trainium2 kernel for nn_Attention_15092515078765.

Sharding: data-parallel over the 32 frames (4 per core) for the VGG13
feature extractor; the sequential attention+GRU decode runs on core 0.

Math shortcut (validated against the reference): the additive-attention
logits are cat([out_t, f]) @ att_w + att_b, and the out_t-dependent term is
identical for every sequence position, so the softmax is invariant to it.
Hence the attention weights -- and the GRU layer-1 input context -- are
constant across all 32 decode steps.
"""

import copy
import json

import numpy as np

import concourse.bass as bass
import concourse.tile as tile
from concourse import mybir
from concourse.bass_utils import run_bass_kernel_spmd

F32 = mybir.dt.float32
AF = mybir.ActivationFunctionType
ALU = mybir.AluOpType

NCORES = 8
FPC = 4            # frames per core
L = 32             # sequence length
PH = 48
EPS = 1e-5

# layer geometry: (cin, cout, H, W); spatial size is at conv input == output
VGG = [
    (3, 64, 160, 96), (64, 64, 160, 96),
    (64, 128, 80, 48), (128, 128, 80, 48),
    (128, 256, 40, 24), (256, 256, 40, 24), (256, 256, 40, 24),
    (256, 512, 20, 12), (512, 512, 20, 12), (512, 512, 20, 12),
    (512, 512, 10, 6), (512, 512, 10, 6), (512, 512, 10, 6),
]

# bias column packing: (li, mb) -> column in the biases array
BIAS_COL = {}
_c = 0
for _li, (_ci, _co, _h, _w) in enumerate(VGG):
    for _mb in range(max(1, _co // 128)):
        BIAS_COL[(_li, _mb)] = _c
        _c += 1
N_BIAS_COL = _c


# --------------------------------------------------------------------------
# TileContext with a walrus-compatible exit: this toolchain's walrus rejects
# >1 sem-wait per instruction, and we avoid the EVSEM butterfly barrier.
# --------------------------------------------------------------------------
class TC(tile.TileContext):
    def _drain_and_barrier(self, tick_clock, wait_clock):
        nc = self.nc
        vc = tick_clock.global_clock
        assert self.sems is not None
        alloc = self.sems.allocated()
        for proc_idx, sem in sorted(alloc.items()):
            tick = vc[proc_idx]
            if tick > 0:
                inc = 16 if 11 <= proc_idx <= 26 else 1
                nc.gpsimd.wait_ge(sem, tick * inc)
        # plain-semaphore all-engine barrier (no EVSEM butterfly, no multi-wait
        # instructions): gpsimd -> done -> every engine acks -> gpsimd clears.
        done = nc.alloc_semaphore("exit_done")
        ack = nc.alloc_semaphore("exit_ack")
        nc.gpsimd.nop().then_inc(done, 1)
        for eng in (nc.tensor, nc.vector, nc.scalar, nc.sync):
            eng.wait_ge(done, 1)
            eng.nop().then_inc(ack, 1)
        nc.gpsimd.wait_ge(ack, 4)
        popped = nc._tile_sem_poison_stack.pop()
        assert popped is self._sem_poison
        nc.clear_and_free_semaphores(list(alloc.values()))
        nc.gpsimd.sem_clear(done)
        nc.gpsimd.sem_clear(ack)

    def _queue_alloc(self, pool):
        # best-fit (not first-fit) gap selection to curb fragmentation
        import concourse.bass as _b
        from concourse._compat import round_up_to_multiple, exact_div
        align = 32
        if self._queue_ring is None:
            ring_base = round_up_to_multiple(self.nc.sbuf_base, align)
            ring_end = self.nc.sbuf_top
            self.nc.sbuf_base = ring_end
            self._queue_ring = (ring_base, ring_end, ring_base)
        ring_base, ring_end, head = self._queue_ring
        k = exact_div(pool.size, self.nc.NUM_PARTITIONS)
        live = sorted(
            la
            for side in ("left", "right")
            for lv in self.pool_stacks[(_b.MemorySpace.SBUF, side)]
            if (la := lv._ring_addr) is not None
        )
        gaps = []
        cur = ring_base
        for lb, le in live:
            if lb > cur:
                gaps.append((cur, lb))
            cur = max(cur, le)
        if ring_end > cur:
            gaps.append((cur, ring_end))
        # static plan for the VGG kernel's pools (lifetime-verified);
        # unknown pools fall back to two-ended placement.
        PLACE = {
            "const": 16512, "tmp": 20160,
            "pin": 36000, "a1": 99552, "a2": 163072, "w01": 178432,
            "p1": 36000, "a3": 101632, "w2": 167264, "w3": 171904,
            "w4": 176544, "a4": 36000, "a4q": 101632, "w5": 119136,
            "a5": 36000, "w6": 137600, "a6": 70976, "w7": 156064,
            "a7": 36000, "a7q": 66752, "w8a": 81888, "a8": 36000,
            "w8b": 118784, "w9a": 155680, "a9": 55744, "w9b": 81888,
            "w10a": 118784, "a10": 36000, "w10b": 81888, "a10q": 51392,
            "w11a": 155680, "a11": 57568, "w11b": 81888, "w12a": 118784,
            "a12": 63744, "w12b": 81888, "a13": 36000, "xf": 39872,
            "stat": 43744,
        }
        if pool.name in PLACE:
            base_addr = PLACE[pool.name]
            end_addr = base_addr + k
            assert end_addr <= ring_end, (pool.name, k, base_addr, ring_end)
            for lb, le in live:
                assert not (base_addr < le and lb < end_addr), (
                    f"plan conflict: {pool.name} [{base_addr},{end_addr}) vs "
                    f"live [{lb},{le})")
            self._queue_ring = (ring_base, ring_end, end_addr)
            pool._ring_addr = (base_addr, end_addr)
            return base_addr, end_addr
        fitting = []
        for gb, ge in gaps:
            gb = round_up_to_multiple(gb, align)
            if ge - gb >= k:
                fitting.append((gb, ge))
        if not fitting:
            raise ValueError(
                f"queue ring full: {pool.name=} ({k}B/part) no gap; "
                f"gaps={[(g[0], g[1] - g[0]) for g in gaps]}")
        if k > 24 * 1024:
            gb, ge = fitting[-1]
            base_addr = (ge - k) // align * align
            assert base_addr >= gb
        else:
            gb, ge = fitting[0]
            base_addr = gb
        end_addr = base_addr + k
        self._queue_ring = (ring_base, ring_end, end_addr)
        pool._ring_addr = (base_addr, end_addr)
        return base_addr, end_addr

    def _process_pool_release(self, pool, inst):
        # allow non-LIFO release for queue-mode pools (ring allocator): the
        # stack is only bookkeeping there; remove by identity instead.
        stack_key = (pool.space, pool.side)
        stack = self.pool_stacks[stack_key]
        if self._is_queue_pool(pool) and (not stack or stack[-1] is not pool):
            for i in range(len(stack) - 1, -1, -1):
                if stack[i] is pool:
                    del stack[i]
                    break
            else:
                raise AssertionError(f"pool {pool.name} not on stack")
            assert pool._ring_addr is not None
            base_addr, end_addr = pool._ring_addr
            self.released_zones[pool.space].append((base_addr, end_addr, inst))
            return
        super()._process_pool_release(pool, inst)


class PoolChain:
    """Open/close tile pools in arbitrary (non-LIFO) order (queue alloc)."""

    def __init__(self, tc):
        self.tc = tc
        self.cms = {}

    def open(self, name, **kw):
        cm = self.tc.tile_pool(name=name, **kw)
        pool = cm.__enter__()
        self.cms[name] = cm
        return pool

    def close(self, name):
        self.cms.pop(name).__exit__(None, None, None)

    def close_all(self):
        for name in reversed(list(self.cms)):
            self.close(name)


def _zero_borders(nc, t, nb, H, W):
    """t: [P, nb, FPC, H+2, W+2] padded activation; zero the 1-px borders."""
    for b in range(nb):
        for f in range(FPC):
            v = t[:, b, f]
            nc.gpsimd.memset(v[:, 0:1, :], 0.0)
            nc.gpsimd.memset(v[:, H + 1:H + 2, :], 0.0)
            nc.gpsimd.memset(v[:, 1:H + 1, 0:1], 0.0)
            nc.gpsimd.memset(v[:, 1:H + 1, W + 1:W + 2], 0.0)


def fix_multi_waits(nc):
    """This walrus pin rejects >1 sem-wait per instruction: hoist extra
    waits onto single-wait EventSemaphore instructions just before."""
    nc.finalize()
    js = json.loads(mybir.module_to_json_string(nc.m))
    template = None
    for f in js["functions"]:
        for bb in f["blocks"]:
            for inst in bb["instructions"]:
                if inst.get("opcode") == "EventSemaphore":
                    template = copy.deepcopy(inst)
                    break
            if template:
                break
        if template:
            break
    assert template is not None, "no EventSemaphore template found"
    n_new = [0]
    n_split = 0
    for f in js["functions"]:
        for bb in f["blocks"]:
            out = []
            for inst in bb["instructions"]:
                si = inst.get("sync_info")
                waits = (si or {}).get("on_wait") or []
                if len(waits) > 1:
                    n_split += 1
                    for w in waits[:-1]:
                        n_new[0] += 1
                        t = copy.deepcopy(template)
                        t["name"] = f"I-mw{n_new[0]}"
                        t["engine"] = inst["engine"]
                        t["sync_info"] = {"on_update": [], "on_wait": [w]}
                        out.append(t)
                    si["on_wait"] = [waits[-1]]
                out.append(inst)
            bb["instructions"] = out
    nc.m = mybir.module_from_json_string(json.dumps(js))
    return n_split


def build_vgg_nc():
    """Per-core program: xs [4,3,160,96] -> feats [4,48]."""
    nc = bass.Bass()
    xs = nc.dram_tensor("xs", [FPC, 27, 162, 98], F32, kind="ExternalInput")
    wd = {0: nc.dram_tensor("w0", [27, 64], F32, kind="ExternalInput")}
    for li in range(1, 13):
        cin, cout, _, _ = VGG[li]
        cinb, coutb = max(1, cin // 128), max(1, cout // 128)
        cinP, coutP = min(cin, 128), min(cout, 128)
        wd[li] = nc.dram_tensor(f"w{li}", [cinP, coutb, cinb, 3, 3, coutP],
                                F32, kind="ExternalInput")
    biases = nc.dram_tensor("biases", [128, N_BIAS_COL], F32, kind="ExternalInput")
    n0w = nc.dram_tensor("n0w", [128, 4, 15], F32, kind="ExternalInput")
    n0b = nc.dram_tensor("n0b", [128, 4, 15], F32, kind="ExternalInput")
    cnnw = nc.dram_tensor("cnnw", [128, 4, 3, 3, 16], F32, kind="ExternalInput")
    cnnb = nc.dram_tensor("cnnb", [16, 1], F32, kind="ExternalInput")
    n1w = nc.dram_tensor("n1w", [16, 3], F32, kind="ExternalInput")
    n1b = nc.dram_tensor("n1b", [16, 3], F32, kind="ExternalInput")
    feats_out = nc.dram_tensor("feats", [FPC, PH], F32, kind="ExternalOutput")
    p1scratch = nc.dram_tensor("p1scratch", [64, FPC, 80, 48], F32)

    with TC(nc, pool_alloc_mode="queue") as tc:
        ch = PoolChain(tc)
        psum = ch.open("ps", bufs=6, space="PSUM")
        tmpp = ch.open("tmp", bufs=1)
        const = ch.open("const", bufs=1)

        bias_sb = const.tile([128, N_BIAS_COL], F32)
        nc.gpsimd.dma_start(out=bias_sb[:, :], in_=biases[:, :])
        n0w_sb = const.tile([128, 4, 15], F32)
        nc.gpsimd.dma_start(out=n0w_sb[:, :, :], in_=n0w[:, :, :])
        n0b_sb = const.tile([128, 4, 15], F32)
        nc.gpsimd.dma_start(out=n0b_sb[:, :, :], in_=n0b[:, :, :])
        cnnw_sb = const.tile([128, 4, 3, 3, 16], F32)
        nc.gpsimd.dma_start(out=cnnw_sb[:, :, :, :, :], in_=cnnw[:, :, :, :, :])
        cnnb_sb = const.tile([16, 1], F32)
        nc.gpsimd.dma_start(out=cnnb_sb[:, :], in_=cnnb[:, :])
        n1w_sb = const.tile([16, 3], F32)
        nc.gpsimd.dma_start(out=n1w_sb[:, :], in_=n1w[:, :])
        n1b_sb = const.tile([16, 3], F32)
        nc.gpsimd.dma_start(out=n1b_sb[:, :], in_=n1b[:, :])
        eps_sb = const.tile([1, 1], F32)
        nc.vector.memset(eps_sb[:, :], EPS)
        ones_row = const.tile([1, 128], F32)
        nc.vector.memset(ones_row[:, :], 1.0)

        def bias_ap(li, mb, coutP):
            col = BIAS_COL[(li, mb)]
            return bias_sb[0:coutP, col:col + 1]

        def load_w(li):
            cin, cout, _, _ = VGG[li]
            cinb, coutb = max(1, cin // 128), max(1, cout // 128)
            cinP, coutP = min(cin, 128), min(cout, 128)
            p = ch.open(f"w{li}", bufs=1)
            t = p.tile([cinP, coutb, cinb, 3, 3, coutP], F32)
            nc.gpsimd.dma_start(out=t[:, :, :, :, :, :], in_=wd[li][:, :, :, :, :, :])
            return t

        def load_w_half(li, half):
            cin, cout, _, _ = VGG[li]
            cinb = max(1, cin // 128)
            cinP = min(cin, 128)
            p = ch.open(f"w{li}{'ab'[half]}", bufs=1)
            t = p.tile([cinP, 2, cinb, 3, 3, 128], F32)
            nc.gpsimd.dma_start(
                out=t[:, :, :, :, :, :],
                in_=wd[li][:, 2 * half:2 * half + 2, :, :, :, :])
            return t

        def close_w_half(li):
            ch.close(f"w{li}a"); ch.close(f"w{li}b")

        # ================= stage A: conv1 + conv2 + pool1, per frame ========
        w01p = ch.open("w01", bufs=1)
        w0_sb = w01p.tile([27, 64], F32)
        nc.gpsimd.dma_start(out=w0_sb[:, :], in_=wd[0][:, :])
        w1_sb = w01p.tile([64, 1, 1, 3, 3, 64], F32)
        nc.gpsimd.dma_start(out=w1_sb[:, :, :, :, :, :], in_=wd[1][:, :, :, :, :, :])

        pinp = ch.open("pin", bufs=1)
        pin = pinp.tile([27, 162, 98], F32)
        a1p = ch.open("a1", bufs=1)
        a1 = a1p.tile([64, 162, 98], F32)
        a2p = ch.open("a2", bufs=1)
        a2 = a2p.tile([64, 40, 96], F32)

        nc.gpsimd.memset(a1[:, 0:1, :], 0.0)
        nc.gpsimd.memset(a1[:, 161:162, :], 0.0)
        nc.gpsimd.memset(a1[:, 1:161, 0:1], 0.0)
        nc.gpsimd.memset(a1[:, 1:161, 97:98], 0.0)

        for f in range(FPC):
            nc.gpsimd.dma_start(out=pin[:, :, :], in_=xs[f, :, :, :])
            # conv1: 32 tiles of 5 rows, K=27 via im2col
            for i in range(32):
                ps = psum.tile([64, 5, 96], F32, tag="ps")
                nc.tensor.matmul(ps[:, :, :], w0_sb[:, :],
                                 pin[:, 1 + 5 * i:6 + 5 * i, 1:97],
                                 start=True, stop=True)
                nc.scalar.activation(out=a1[:, 1 + 5 * i:6 + 5 * i, 1:97],
                                     in_=ps[:, :, :], func=AF.Relu,
                                     bias=bias_ap(0, 0, 64), scale=1.0)
            # conv2 (9 accumulated taps) in 4 chunks of 40 rows, + pool1
            for c4 in range(4):
                for i in range(8):
                    r0 = c4 * 40 + i * 5
                    ps = psum.tile([64, 5, 96], F32, tag="ps")
                    for t9 in range(9):
                        ti, tj = t9 // 3, t9 % 3
                        nc.tensor.matmul(ps[:, :, :], w1_sb[:, 0, 0, ti, tj, :],
                                         a1[:, r0 + ti:r0 + ti + 5, tj:tj + 96],
                                         start=(t9 == 0), stop=(t9 == 8))
                    nc.scalar.activation(out=a2[:, 5 * i:5 * i + 5, :],
                                         in_=ps[:, :, :], func=AF.Relu,
                                         bias=bias_ap(1, 0, 64), scale=1.0)
                for hh in range(2):
                    tmp = tmpp.tile([64, 20, 48], F32, tag="tmpA")
                    v = a2[:, 20 * hh:20 * hh + 20, :].rearrange(
                        "p h (w two) -> p h w two", two=2)
                    nc.vector.tensor_max(tmp[:, :, :], v[:, :, :, 0], v[:, :, :, 1])
                    pout = tmpp.tile([64, 10, 48], F32, tag="poolout")
                    u = tmp.rearrange("p (h two) w -> p h two w", two=2)
                    nc.vector.tensor_max(pout[:, :, :], u[:, :, 0, :], u[:, :, 1, :])
                    nc.gpsimd.dma_start(
                        out=p1scratch[:, f, 20 * c4 + 10 * hh:20 * c4 + 10 * hh + 10, :],
                        in_=pout[:, :, :])

        ch.close("pin"); ch.close("a1"); ch.close("a2"); ch.close("w01")
        p1pool = ch.open("p1", bufs=1)
        p1 = p1pool.tile([64, 1, FPC, 82, 50], F32)
        _zero_borders(nc, p1, 1, 80, 48)
        for f in range(FPC):
            nc.gpsimd.dma_start(out=p1[:, 0, f, 1:81, 1:49],
                                in_=p1scratch[:, f, :, :])

        # ================= stage B: conv3..conv13, 4 frames batched =========
        def conv_b(src, dst, li, w_sb, rows_per_tile, dst_padded):
            cin, cout, H, W = VGG[li]
            cinb, coutb = max(1, cin // 128), max(1, cout // 128)
            cinP, coutP = min(cin, 128), min(cout, 128)
            for mb in range(coutb):
                if isinstance(w_sb, tuple):
                    wt, mbl = w_sb[mb // 2], mb % 2
                else:
                    wt, mbl = w_sb, mb
                for r0 in range(0, H, rows_per_tile):
                    rows = min(rows_per_tile, H - r0)
                    ps = psum.tile([coutP, FPC, rows, W], F32, tag="ps")
                    n = 0
                    for cb in range(cinb):
                        for t9 in range(9):
                            ti, tj = t9 // 3, t9 % 3
                            nc.tensor.matmul(
                                ps[:, :, :, :], wt[:, mbl, cb, ti, tj, :],
                                src[:, cb, :, r0 + ti:r0 + ti + rows, tj:tj + W],
                                start=(n == 0), stop=(n == cinb * 9 - 1))
                            n += 1
                    if dst_padded:
                        out = dst[:, mb, :, 1 + r0:1 + r0 + rows, 1:W + 1]
                    else:
                        out = dst[:, mb, :, r0:r0 + rows, :]
                    nc.scalar.activation(out=out, in_=ps[:, :, :, :], func=AF.Relu,
                                         bias=bias_ap(li, mb, coutP), scale=1.0)

        def maxpool_b(src, dst, nb, H, W):
            Ho, Wo = H // 2, W // 2
            for b in range(nb):
                for f in range(FPC):
                    tmp = tmpp.tile([128, H, Wo], F32, tag="tmpB")
                    v = src[:, b, f].rearrange("p h (w two) -> p h w two", two=2)
                    nc.vector.tensor_max(tmp[:, :, :], v[:, :, :, 0], v[:, :, :, 1])
                    u = tmp.rearrange("p (h two) w -> p h two w", two=2)
                    nc.vector.tensor_max(dst[:, b, f, 1:Ho + 1, 1:Wo + 1],
                                         u[:, :, 0, :], u[:, :, 1, :])

        def new_act(name, nb, H, W, padded=True):
            p = ch.open(name, bufs=1)
            if padded:
                t = p.tile([128, nb, FPC, H + 2, W + 2], F32)
                _zero_borders(nc, t, nb, H, W)
            else:
                t = p.tile([128, nb, FPC, H, W], F32)
            return t

        w2 = load_w(2); w3 = load_w(3)
        a3 = new_act("a3", 1, 80, 48)
        conv_b(p1, a3, 2, w2, 2, True)
        ch.close("p1"); ch.close("w2")
        w4 = load_w(4)
        a4 = new_act("a4", 1, 80, 48, padded=False)
        conv_b(a3, a4, 3, w3, 2, False)
        ch.close("a3"); ch.close("w3")
        a4q = new_act("a4q", 1, 40, 24)
        maxpool_b(a4, a4q, 1, 80, 48)
        ch.close("a4")
        w5 = load_w(5)
        a5 = new_act("a5", 2, 40, 24)
        conv_b(a4q, a5, 4, w4, 5, True)
        ch.close("a4q"); ch.close("w4")
        w6 = load_w(6)
        a6 = new_act("a6", 2, 40, 24)
        conv_b(a5, a6, 5, w5, 5, True)
        ch.close("a5"); ch.close("w5")
        w7 = load_w(7)
        a7 = new_act("a7", 2, 40, 24, padded=False)
        conv_b(a6, a7, 6, w6, 5, False)
        ch.close("a6"); ch.close("w6")
        a7q = new_act("a7q", 2, 20, 12)
        maxpool_b(a7, a7q, 2, 40, 24)
        ch.close("a7")
        w8a = load_w_half(8, 0)
        a8 = new_act("a8", 4, 20, 12)
        conv_b(a7q, a8, 7, w7, 9, True)
        w8b = load_w_half(8, 1)
        ch.close("a7q"); ch.close("w7")
        w9a = load_w_half(9, 0)
        a9 = new_act("a9", 4, 20, 12)
        conv_b(a8, a9, 8, (w8a, w8b), 9, True)
        ch.close("a8"); close_w_half(8)
        w9b = load_w_half(9, 1)
        w10a = load_w_half(10, 0)
        a10 = new_act("a10", 4, 20, 12, padded=False)
        conv_b(a9, a10, 9, (w9a, w9b), 9, False)
        ch.close("a9"); close_w_half(9)
        w10b = load_w_half(10, 1)
        a10q = new_act("a10q", 4, 10, 6)
        maxpool_b(a10, a10q, 4, 20, 12)
        ch.close("a10")
        w11a = load_w_half(11, 0)
        a11 = new_act("a11", 4, 10, 6)
        conv_b(a10q, a11, 10, (w10a, w10b), 10, True)
        ch.close("a10q"); close_w_half(10)
        w11b = load_w_half(11, 1)
        w12a = load_w_half(12, 0)
        a12 = new_act("a12", 4, 10, 6)
        conv_b(a11, a12, 11, (w11a, w11b), 10, True)
        ch.close("a11"); close_w_half(11)
        w12b = load_w_half(12, 1)
        a13 = new_act("a13", 4, 10, 6, padded=False)
        conv_b(a12, a13, 12, (w12a, w12b), 10, False)
        ch.close("a12"); close_w_half(12)

        # pool5 -> xf [128, 4, FPC, 5, 3]
        xfp = ch.open("xf", bufs=1)
        xf = xfp.tile([128, 4, FPC, 5, 3], F32)
        for b in range(4):
            for f in range(FPC):
                tmp = tmpp.tile([128, 10, 3], F32, tag="tmpB")
                v = a13[:, b, f].rearrange("p h (w two) -> p h w two", two=2)
                nc.vector.tensor_max(tmp[:, :, :], v[:, :, :, 0], v[:, :, :, 1])
                u = tmp.rearrange("p (h two) w -> p h two w", two=2)
                nc.vector.tensor_max(xf[:, b, f, :, :], u[:, :, 0, :], u[:, :, 1, :])
        ch.close("a13")

        # ---------------- norm0 (LN over 7680 per frame) --------------------
        statp = ch.open("stat", bufs=1)
        sq = statp.tile([128, 4, FPC, 15], F32)
        xfv = xf.rearrange("p b f h w -> p b f (h w)")
        nc.vector.tensor_mul(sq[:, :, :, :], xfv, xfv)
        colsum = statp.tile([1, 4, FPC, 15], F32)
        colsq = statp.tile([1, 4, FPC, 15], F32)
        nc.gpsimd.tensor_reduce(colsum[:, :, :, :], xfv,
                                axis=mybir.AxisListType.C, op=ALU.add)
        nc.gpsimd.tensor_reduce(colsq[:, :, :, :], sq[:, :, :, :],
                                axis=mybir.AxisListType.C, op=ALU.add)
        fsum = statp.tile([1, FPC, 2], F32)
        for f in range(FPC):
            nc.vector.tensor_reduce(fsum[:, f, 0:1], colsum[:, :, f, :],
                                    axis=mybir.AxisListType.XY, op=ALU.add)
            nc.vector.tensor_reduce(fsum[:, f, 1:2], colsq[:, :, f, :],
                                    axis=mybir.AxisListType.XY, op=ALU.add)
        mv = statp.tile([1, FPC, 3], F32)          # mu, rstd, -mu
        nc.scalar.mul(mv[:, :, 0:1], fsum[:, :, 0:1], 1.0 / 7680.0)
        musq = statp.tile([1, FPC, 1], F32)
        nc.vector.tensor_mul(musq[:, :, :], mv[:, :, 0:1], mv[:, :, 0:1])
        var = statp.tile([1, FPC, 1], F32)
        nc.vector.scalar_tensor_tensor(var[:, :, :], fsum[:, :, 1:2], 1.0 / 7680.0,
                                       musq[:, :, :], op0=ALU.mult,
                                       op1=ALU.subtract)
        sd = statp.tile([1, FPC, 1], F32)
        nc.scalar.activation(sd[:, :, :], var[:, :, :], AF.Sqrt, bias=eps_sb[:, 0:1], scale=1.0)
        nc.vector.reciprocal(mv[:, :, 1:2], sd[:, :, :])
        nc.scalar.mul(mv[:, :, 2:3], mv[:, :, 0:1], -1.0)
        bc = statp.tile([128, FPC, 2], F32)        # [...,0]=rstd, [...,1]=-mu
        ps_b = psum.tile([128, FPC, 2], F32, tag="ps")
        nc.tensor.matmul(ps_b[:, :, :], ones_row[:, :], mv[:, :, 1:3],
                         start=True, stop=True)
        nc.vector.tensor_copy(bc[:, :, :], ps_b[:, :, :])
        xn = statp.tile([128, 4, FPC, 15], F32)
        for f in range(FPC):
            nc.vector.tensor_scalar(out=xn[:, :, f, :], in0=xfv[:, :, f, :],
                                    scalar1=bc[:, f, 1:2], scalar2=bc[:, f, 0:1],
                                    op0=ALU.add, op1=ALU.mult)
            nc.vector.tensor_mul(xn[:, :, f, :], xn[:, :, f, :], n0w_sb[:, :, :])
            nc.vector.tensor_add(xn[:, :, f, :], xn[:, :, f, :], n0b_sb[:, :, :])

        # ---------------- cnn 512->16 (3x3 valid on 5x3 -> 3x1), leaky ------
        xn5 = xn.rearrange("p b f (h w) -> p b f h w", w=3)
        psc = psum.tile([16, FPC, 3], F32, tag="ps")
        n = 0
        for cb in range(4):
            for ti in range(3):
                for tj in range(3):
                    nc.tensor.matmul(psc[:, :, :], cnnw_sb[:, cb, ti, tj, :],
                                     xn5[:, cb, :, ti:ti + 3, tj:tj + 1],
                                     start=(n == 0), stop=(n == 35))
                    n += 1
        fpre = statp.tile([16, FPC, 3], F32)
        xb = statp.tile([16, FPC, 3], F32)
        nc.vector.tensor_scalar_add(xb[:, :, :], in0=psc[:, :, :],
                                    scalar1=cnnb_sb[:, 0:1])
        xmn = statp.tile([16, FPC, 3], F32)
        nc.vector.tensor_scalar_min(xmn[:, :, :], in0=xb[:, :, :], scalar1=0.0)
        xmx = statp.tile([16, FPC, 3], F32)
        nc.vector.tensor_scalar_max(xmx[:, :, :], in0=xb[:, :, :], scalar1=0.0)
        nc.vector.scalar_tensor_tensor(fpre[:, :, :], xmn[:, :, :], 0.01,
                                       xmx[:, :, :], op0=ALU.mult, op1=ALU.add)

        # ---------------- norm1 (LN over 48 per frame) ----------------------
        sq1 = statp.tile([16, FPC, 3], F32)
        nc.vector.tensor_mul(sq1[:, :, :], fpre[:, :, :], fpre[:, :, :])
        c1 = statp.tile([1, FPC, 3], F32)
        c2 = statp.tile([1, FPC, 3], F32)
        nc.gpsimd.tensor_reduce(c1[:, :, :], fpre[:, :, :],
                                axis=mybir.AxisListType.C, op=ALU.add)
        nc.gpsimd.tensor_reduce(c2[:, :, :], sq1[:, :, :],
                                axis=mybir.AxisListType.C, op=ALU.add)
        f1 = statp.tile([1, FPC, 2], F32)
        for f in range(FPC):
            nc.vector.tensor_reduce(f1[:, f, 0:1], c1[:, f, :],
                                    axis=mybir.AxisListType.X, op=ALU.add)
            nc.vector.tensor_reduce(f1[:, f, 1:2], c2[:, f, :],
                                    axis=mybir.AxisListType.X, op=ALU.add)
        mv1 = statp.tile([1, FPC, 3], F32)
        nc.scalar.mul(mv1[:, :, 0:1], f1[:, :, 0:1], 1.0 / 48.0)
        musq1 = statp.tile([1, FPC, 1], F32)
        nc.vector.tensor_mul(musq1[:, :, :], mv1[:, :, 0:1], mv1[:, :, 0:1])
        var1 = statp.tile([1, FPC, 1], F32)
        nc.vector.scalar_tensor_tensor(var1[:, :, :], f1[:, :, 1:2], 1.0 / 48.0,
                                       musq1[:, :, :], op0=ALU.mult,
                                       op1=ALU.subtract)
        sd1 = statp.tile([1, FPC, 1], F32)
        nc.scalar.activation(sd1[:, :, :], var1[:, :, :], AF.Sqrt,
                             bias=eps_sb[:, 0:1], scale=1.0)
        nc.vector.reciprocal(mv1[:, :, 1:2], sd1[:, :, :])
        nc.scalar.mul(mv1[:, :, 2:3], mv1[:, :, 0:1], -1.0)
        bc1 = statp.tile([16, FPC, 2], F32)        # [...,0]=rstd, [...,1]=-mu
        ps_b1 = psum.tile([16, FPC, 2], F32, tag="ps")
        nc.tensor.matmul(ps_b1[:, :, :], ones_row[0:1, 0:16], mv1[:, :, 1:3],
                         start=True, stop=True)
        nc.vector.tensor_copy(bc1[:, :, :], ps_b1[:, :, :])
        feats_sb = statp.tile([16, FPC, 3], F32)
        for f in range(FPC):
            nc.vector.tensor_scalar(out=feats_sb[:, f, :], in0=fpre[:, f, :],
                                    scalar1=bc1[:, f, 1:2], scalar2=bc1[:, f, 0:1],
                                    op0=ALU.add, op1=ALU.mult)
            nc.vector.tensor_mul(feats_sb[:, f, :], feats_sb[:, f, :], n1w_sb[:, :])
            nc.vector.tensor_add(feats_sb[:, f, :], feats_sb[:, f, :], n1b_sb[:, :])
        nc.gpsimd.dma_start(
            out=feats_out[:, :].rearrange("f (c p) -> c f p", p=3),
            in_=feats_sb[:, :, :])
        ch.close_all()
    return nc


def build_decode_nc():
    """Core-0 program: feats [32,48] + recurrent weights -> o [1,11], hf [2,1,48]."""
    nc = bass.Bass()
    f_d = nc.dram_tensor("f", [L, PH], F32, kind="ExternalInput")
    fT_d = nc.dram_tensor("fT", [PH, L], F32, kind="ExternalInput")
    w2_d = nc.dram_tensor("w2col", [PH, 1], F32, kind="ExternalInput")
    wih1_d = nc.dram_tensor("wih1T", [PH, 3 * PH], F32, kind="ExternalInput")
    whh1_d = nc.dram_tensor("whh1T", [PH, 3 * PH], F32, kind="ExternalInput")
    wih2_d = nc.dram_tensor("wih2T", [PH, 3 * PH], F32, kind="ExternalInput")
    whh2_d = nc.dram_tensor("whh2T", [PH, 3 * PH], F32, kind="ExternalInput")
    dc_d = nc.dram_tensor("dc", [PH, 8], F32, kind="ExternalInput")
    h0_d = nc.dram_tensor("h0", [PH, 2], F32, kind="ExternalInput")
    prew_d = nc.dram_tensor("prewT", [PH, 16], F32, kind="ExternalInput")
    clsw_d = nc.dram_tensor("clswT", [16, 11], F32, kind="ExternalInput")
    hc_d = nc.dram_tensor("hc", [16, 6], F32, kind="ExternalInput")
    o_out = nc.dram_tensor("o", [1, 11], F32, kind="ExternalOutput")
    hf_out = nc.dram_tensor("hf", [2, PH], F32, kind="ExternalOutput")

    with TC(nc, pool_alloc_mode="queue") as tc:
        ch = PoolChain(tc)
        ps = ch.open("ps", bufs=8, space="PSUM")
        sb = ch.open("sb", bufs=1)
        st = ch.open("st", bufs=2)

        f_sb = sb.tile([L, PH], F32)
        nc.gpsimd.dma_start(out=f_sb[:, :], in_=f_d[:, :])
        fT_sb = sb.tile([PH, L], F32)
        nc.gpsimd.dma_start(out=fT_sb[:, :], in_=fT_d[:, :])
        w2_sb = sb.tile([PH, 1], F32)
        nc.gpsimd.dma_start(out=w2_sb[:, :], in_=w2_d[:, :])
        wih1 = sb.tile([PH, 3 * PH], F32)
        nc.gpsimd.dma_start(out=wih1[:, :], in_=wih1_d[:, :])
        whh1 = sb.tile([PH, 3 * PH], F32)
        nc.gpsimd.dma_start(out=whh1[:, :], in_=whh1_d[:, :])
        wih2 = sb.tile([PH, 3 * PH], F32)
        nc.gpsimd.dma_start(out=wih2[:, :], in_=wih2_d[:, :])
        whh2 = sb.tile([PH, 3 * PH], F32)
        nc.gpsimd.dma_start(out=whh2[:, :], in_=whh2_d[:, :])
        dc = sb.tile([PH, 8], F32)
        nc.gpsimd.dma_start(out=dc[:, :], in_=dc_d[:, :])
        h0 = sb.tile([PH, 2], F32)
        nc.gpsimd.dma_start(out=h0[:, :], in_=h0_d[:, :])
        prew = sb.tile([PH, 16], F32)
        nc.gpsimd.dma_start(out=prew[:, :], in_=prew_d[:, :])
        clsw = sb.tile([16, 11], F32)
        nc.gpsimd.dma_start(out=clsw[:, :], in_=clsw_d[:, :])
        hc = sb.tile([16, 6], F32)
        nc.gpsimd.dma_start(out=hc[:, :], in_=hc_d[:, :])
        eps16 = sb.tile([16, 1], F32)
        nc.vector.memset(eps16[:, :], EPS)
        ones_c = sb.tile([16, 1], F32)
        nc.vector.memset(ones_c[:, :], 1.0)
        ones_r = sb.tile([1, 16], F32)
        nc.vector.memset(ones_r[:, :], 1.0)

        # attention (constant across steps): a = softmax(f @ w2); ctx = a @ f
        ps_l = ps.tile([1, L], F32, tag="pp")
        nc.tensor.matmul(ps_l[:, :], w2_sb[:, :], fT_sb[:, :], start=True, stop=True)
        mx = sb.tile([1, 1], F32)
        nc.vector.reduce_max(mx[:, :], ps_l[:, :], axis=mybir.AxisListType.X)
        negm = sb.tile([1, 1], F32)
        nc.scalar.mul(negm[:, :], mx[:, :], -1.0)
        e = sb.tile([1, L], F32)
        esum = sb.tile([1, 1], F32)
        nc.scalar.activation(e[:, :], ps_l[:, :], AF.Exp, bias=negm[:, 0:1],
                             scale=1.0, accum_out=esum[:, :])
        rs = sb.tile([1, 1], F32)
        nc.vector.reciprocal(rs[:, :], esum[:, :])
        a_row = sb.tile([1, L], F32)
        nc.vector.tensor_scalar_mul(a_row[:, :], in0=e[:, :], scalar1=rs[:, 0:1])
        a_col = sb.tile([L, 1], F32)
        nc.gpsimd.dma_start(out=a_col[:, :], in_=a_row[:, :])
        ps_c = ps.tile([PH, 1], F32, tag="pp")
        nc.tensor.matmul(ps_c[:, :], f_sb[:, :], a_col[:, :], start=True, stop=True)
        ctx = sb.tile([PH, 1], F32)
        nc.vector.tensor_copy(ctx[:, :], ps_c[:, :])

        # gi1 = W_ih1 @ ctx (+ gate-combined biases), constant across steps
        gib = sb.tile([PH, 3], F32)      # cols: r-bias, z-bias, n-part
        for g in range(3):
            pg = ps.tile([PH, 1], F32, tag="pp")
            nc.tensor.matmul(pg[:, :], wih1[:, 48 * g:48 * g + 48], ctx[:, :],
                             start=True, stop=True)
            nc.vector.tensor_scalar_add(gib[:, g:g + 1], in0=pg[:, :],
                                        scalar1=dc[:, 1 + g:2 + g])
        h1 = sb.tile([PH, 1], F32)
        nc.vector.tensor_copy(h1[:, :], h0[:, 0:1])
        h2 = sb.tile([PH, 1], F32)
        nc.vector.tensor_copy(h2[:, :], h0[:, 1:2])

        for step in range(L):
            p_r1 = ps.tile([PH, 1], F32, tag="pp")
            p_z1 = ps.tile([PH, 1], F32, tag="pp")
            p_n1 = ps.tile([PH, 1], F32, tag="pp")
            nc.tensor.matmul(p_r1[:, :], whh1[:, 0:48], h1[:, :], start=True, stop=True)
            nc.tensor.matmul(p_z1[:, :], whh1[:, 48:96], h1[:, :], start=True, stop=True)
            nc.tensor.matmul(p_n1[:, :], whh1[:, 96:144], h1[:, :], start=True, stop=True)
            r1 = st.tile([PH, 1], F32, tag="r1")
            nc.scalar.activation(r1[:, :], p_r1[:, :], AF.Sigmoid,
                                 bias=gib[:, 0:1], scale=1.0)
            z1 = st.tile([PH, 1], F32, tag="z1")
            nc.scalar.activation(z1[:, :], p_z1[:, :], AF.Sigmoid,
                                 bias=gib[:, 1:2], scale=1.0)
            t1 = st.tile([PH, 1], F32, tag="t1")
            nc.vector.scalar_tensor_tensor(t1[:, :], p_n1[:, :], dc[:, 0:1],
                                           r1[:, :], op0=ALU.add, op1=ALU.mult)
            n1 = st.tile([PH, 1], F32, tag="n1")
            nc.scalar.activation(n1[:, :], t1[:, :], AF.Tanh,
                                 bias=gib[:, 2:3], scale=1.0)
            d1 = st.tile([PH, 1], F32, tag="d1")
            nc.vector.tensor_sub(d1[:, :], h1[:, :], n1[:, :])
            h1n = st.tile([PH, 1], F32, tag="h1n")
            nc.vector.scalar_tensor_tensor(h1n[:, :], d1[:, :], z1[:, 0:1],
                                           n1[:, :], op0=ALU.mult, op1=ALU.add)
            h1 = h1n

            p_r2 = ps.tile([PH, 1], F32, tag="pp")
            p_z2 = ps.tile([PH, 1], F32, tag="pp")
            p_gn2 = ps.tile([PH, 1], F32, tag="pp")
            p_hn2 = ps.tile([PH, 1], F32, tag="pp")
            nc.tensor.matmul(p_r2[:, :], wih2[:, 0:48], h1[:, :], start=True, stop=False)
            nc.tensor.matmul(p_r2[:, :], whh2[:, 0:48], h2[:, :], start=False, stop=True)
            nc.tensor.matmul(p_z2[:, :], wih2[:, 48:96], h1[:, :], start=True, stop=False)
            nc.tensor.matmul(p_z2[:, :], whh2[:, 48:96], h2[:, :], start=False, stop=True)
            nc.tensor.matmul(p_gn2[:, :], wih2[:, 96:144], h1[:, :], start=True, stop=True)
            nc.tensor.matmul(p_hn2[:, :], whh2[:, 96:144], h2[:, :], start=True, stop=True)
            r2 = st.tile([PH, 1], F32, tag="r2")
            nc.scalar.activation(r2[:, :], p_r2[:, :], AF.Sigmoid,
                                 bias=dc[:, 4:5], scale=1.0)
            z2 = st.tile([PH, 1], F32, tag="z2")
            nc.scalar.activation(z2[:, :], p_z2[:, :], AF.Sigmoid,
                                 bias=dc[:, 5:6], scale=1.0)
            t2 = st.tile([PH, 1], F32, tag="t2")
            nc.vector.scalar_tensor_tensor(t2[:, :], p_hn2[:, :], dc[:, 7:8],
                                           r2[:, :], op0=ALU.add, op1=ALU.mult)
            s2 = st.tile([PH, 1], F32, tag="s2")
            nc.vector.scalar_tensor_tensor(s2[:, :], p_gn2[:, :], dc[:, 6:7],
                                           t2[:, :], op0=ALU.add, op1=ALU.add)
            n2 = st.tile([PH, 1], F32, tag="n2")
            nc.scalar.activation(n2[:, :], s2[:, :], AF.Tanh, bias=0.0, scale=1.0)
            d2 = st.tile([PH, 1], F32, tag="d2")
            nc.vector.tensor_sub(d2[:, :], h2[:, :], n2[:, :])
            h2n = st.tile([PH, 1], F32, tag="h2n")
            nc.vector.scalar_tensor_tensor(h2n[:, :], d2[:, :], z2[:, 0:1],
                                           n2[:, :], op0=ALU.mult, op1=ALU.add)
            h2 = h2n

        # head: o = LN(leaky(h2 @ pre_w.T + pre_b)); o = LN(o @ cls_w.T + cls_b)
        def psum_all(x, nchan, tag):
            p1 = ps.tile([1, 1], F32, tag="pp")
            nc.tensor.matmul(p1[:, :], ones_c[0:nchan, :], x[0:nchan, :],
                             start=True, stop=True)
            s1 = st.tile([1, 1], F32, tag=tag + "s")
            nc.vector.tensor_copy(s1[:, :], p1[:, :])
            p2 = ps.tile([16, 1], F32, tag="pp")
            nc.tensor.matmul(p2[0:nchan, :], ones_r[0:1, 0:nchan], s1[:, :],
                             start=True, stop=True)
            return p2

        def col_ln(x, nchan, w_ap, b_ap):
            red = psum_all(x, nchan, "lnr")
            mu = st.tile([16, 1], F32, tag="lnm")
            nc.scalar.mul(mu[0:nchan, :], red[0:nchan, :], 1.0 / nchan)
            xc = st.tile([16, 1], F32, tag="lnx")
            nc.vector.tensor_sub(xc[0:nchan, :], x[0:nchan, :], mu[0:nchan, :])
            sqc = st.tile([16, 1], F32, tag="lns")
            nc.vector.tensor_mul(sqc[0:nchan, :], xc[0:nchan, :], xc[0:nchan, :])
            sv = psum_all(sqc, nchan, "lnv")
            sdv = st.tile([16, 1], F32, tag="lnd")
            nc.scalar.activation(sdv[0:nchan, :], sv[0:nchan, :], AF.Sqrt,
                                 bias=eps16[0:nchan, 0:1], scale=1.0 / nchan)
            rsv = st.tile([16, 1], F32, tag="lne")
            nc.vector.reciprocal(rsv[0:nchan, :], sdv[0:nchan, :])
            y = st.tile([16, 1], F32, tag="lny")
            nc.vector.tensor_mul(y[0:nchan, :], xc[0:nchan, :], rsv[0:nchan, :])
            nc.vector.tensor_mul(y[0:nchan, :], y[0:nchan, :], w_ap)
            nc.vector.tensor_add(y[0:nchan, :], y[0:nchan, :], b_ap)
            return y

        p_o1 = ps.tile([16, 1], F32, tag="pp")
        nc.tensor.matmul(p_o1[:, :], prew[:, :], h2[:, :], start=True, stop=True)
        o1 = st.tile([16, 1], F32, tag="o1")
        o1b = st.tile([16, 1], F32, tag="o1b")
        nc.vector.tensor_scalar_add(o1b[:, :], in0=p_o1[:, :], scalar1=hc[:, 0:1])
        o1mn = st.tile([16, 1], F32, tag="o1mn")
        nc.vector.tensor_scalar_min(o1mn[:, :], in0=o1b[:, :], scalar1=0.0)
        o1mx = st.tile([16, 1], F32, tag="o1mx")
        nc.vector.tensor_scalar_max(o1mx[:, :], in0=o1b[:, :], scalar1=0.0)
        nc.vector.scalar_tensor_tensor(o1[:, :], o1mn[:, :], 0.01,
                                       o1mx[:, :], op0=ALU.mult, op1=ALU.add)
        y1 = col_ln(o1, 16, hc[0:16, 1:2], hc[0:16, 2:3])
        p_o2 = ps.tile([11, 1], F32, tag="pp")
        nc.tensor.matmul(p_o2[:, :], clsw[:, :], y1[0:16, :], start=True, stop=True)
        o2 = st.tile([16, 1], F32, tag="o2")
        nc.vector.tensor_scalar_add(o2[0:11, :], in0=p_o2[:, :],
                                    scalar1=hc[0:11, 3:4])
        y2 = col_ln(o2, 11, hc[0:11, 4:5], hc[0:11, 5:6])

        nc.gpsimd.dma_start(out=o_out[0, :], in_=y2[0:11, :])
        nc.gpsimd.dma_start(out=hf_out[0, :], in_=h1[:, :])
        nc.gpsimd.dma_start(out=hf_out[1, :], in_=h2[:, :])
        ch.close_all()
    return nc


# ============================ host-side glue ===============================

def im2col_x(xs):
    """xs [N,3,160,96] -> [N,27,162,98]: row 3t+c = channel c shifted so a
    fixed-offset read yields tap t of a pad-1 3x3 conv."""
    n = xs.shape[0]
    out = np.zeros((n, 27, 162, 98), np.float32)
    for t in range(9):
        ti, tj = t // 3, t % 3
        r0, c0 = 2 - ti, 2 - tj
        out[:, 3 * t:3 * t + 3, r0:r0 + 160, c0:c0 + 96] = xs
    return out


def _prep_vgg_weights(vgg_params, norm0_w, norm0_b, cnn_w, cnn_b, norm1_w, norm1_b):
    d = {}
    W0, b0 = vgg_params[0]
    W0 = np.asarray(W0, np.float32)
    d["w0"] = np.ascontiguousarray(W0.transpose(2, 3, 1, 0).reshape(27, 64))
    for li in range(1, 13):
        W, _ = vgg_params[li]
        W = np.asarray(W, np.float32)
        cin, cout, _, _ = VGG[li]
        cinb, coutb = max(1, cin // 128), max(1, cout // 128)
        cinP, coutP = min(cin, 128), min(cout, 128)
        arr = W.reshape(coutb, coutP, cinb, cinP, 3, 3).transpose(3, 0, 2, 4, 5, 1)
        d[f"w{li}"] = np.ascontiguousarray(arr)
    biases = np.zeros((128, N_BIAS_COL), np.float32)
    for li in range(13):
        b = np.asarray(vgg_params[li][1], np.float32)
        cout = VGG[li][1]
        coutP = min(cout, 128)
        for mb in range(max(1, cout // 128)):
            biases[0:coutP, BIAS_COL[(li, mb)]] = b[mb * 128:mb * 128 + coutP]
    d["biases"] = biases
    d["n0w"] = np.ascontiguousarray(
        np.asarray(norm0_w, np.float32).reshape(4, 128, 15).transpose(1, 0, 2))
    d["n0b"] = np.ascontiguousarray(
        np.asarray(norm0_b, np.float32).reshape(4, 128, 15).transpose(1, 0, 2))
    cw = np.asarray(cnn_w, np.float32)           # [16, 512, 3, 3]
    d["cnnw"] = np.ascontiguousarray(
        cw.reshape(16, 4, 128, 3, 3).transpose(2, 1, 3, 4, 0))
    d["cnnb"] = np.asarray(cnn_b, np.float32).reshape(16, 1)
    d["n1w"] = np.asarray(norm1_w, np.float32).reshape(16, 3)
    d["n1b"] = np.asarray(norm1_b, np.float32).reshape(16, 3)
    return d


def _prep_decode(feats, previous_state, att_w, gru_params, pre_w, pre_b,
                 norm2_w, norm2_b, cls_w, cls_b, norm3_w, norm3_b):
    f = np.asarray(feats, np.float32)
    d = {"f": f, "fT": np.ascontiguousarray(f.T)}
    d["w2col"] = np.ascontiguousarray(
        np.asarray(att_w, np.float32)[0, PH:2 * PH].reshape(PH, 1))
    (wi1, wh1, bi1, bh1), (wi2, wh2, bi2, bh2) = [
        tuple(np.asarray(a, np.float32) for a in g) for g in gru_params]
    d["wih1T"] = np.ascontiguousarray(wi1.T)
    d["whh1T"] = np.ascontiguousarray(wh1.T)
    d["wih2T"] = np.ascontiguousarray(wi2.T)
    d["whh2T"] = np.ascontiguousarray(wh2.T)
    dc = np.zeros((PH, 8), np.float32)
    dc[:, 0] = bh1[96:144]
    dc[:, 1] = bi1[0:48] + bh1[0:48]
    dc[:, 2] = bi1[48:96] + bh1[48:96]
    dc[:, 3] = bi1[96:144]
    dc[:, 4] = bi2[0:48] + bh2[0:48]
    dc[:, 5] = bi2[48:96] + bh2[48:96]
    dc[:, 6] = bi2[96:144]
    dc[:, 7] = bh2[96:144]
    d["dc"] = dc
    d["h0"] = np.ascontiguousarray(
        np.asarray(previous_state, np.float32).reshape(2, PH).T)
    d["prewT"] = np.ascontiguousarray(np.asarray(pre_w, np.float32).T)
    d["clswT"] = np.ascontiguousarray(np.asarray(cls_w, np.float32).T)
    hcol = np.zeros((16, 6), np.float32)
    hcol[0:16, 0] = np.asarray(pre_b, np.float32)
    hcol[0:16, 1] = np.asarray(norm2_w, np.float32)
    hcol[0:16, 2] = np.asarray(norm2_b, np.float32)
    hcol[0:11, 3] = np.asarray(cls_b, np.float32)
    hcol[0:11, 4] = np.asarray(norm3_w, np.float32)
    hcol[0:11, 5] = np.asarray(norm3_b, np.float32)
    d["hc"] = hcol
    return d


_CACHE = {}


def _get_ncs():
    if "vgg" not in _CACHE:
        nc_v = build_vgg_nc()
        fix_multi_waits(nc_v)
        nc_d = build_decode_nc()
        fix_multi_waits(nc_d)
        _CACHE["vgg"] = nc_v
        _CACHE["dec"] = nc_d
    return _CACHE["vgg"], _CACHE["dec"]


def kernel(x, lengths, previous_state, vgg_params, norm0_w, norm0_b, cnn_w, cnn_b,
           norm1_w, norm1_b, att_w, att_b, gru_params, pre_w, pre_b,
           norm2_w, norm2_b, cls_w, cls_b, norm3_w, norm3_b):
    assert int(lengths) == L, f"kernel specialized for lengths={L}"
    x = np.asarray(x, np.float32)
    nc_vgg, nc_dec = _get_ncs()

    import time as _t
    t0 = _t.time()
    wmap = _prep_vgg_weights(vgg_params, norm0_w, norm0_b, cnn_w, cnn_b,
                             norm1_w, norm1_b)
    t1 = _t.time()
    xs = im2col_x(x.reshape(L, 3, 160, 96))
    in_maps = []
    for c in range(NCORES):
        m = dict(wmap)
        m["xs"] = np.ascontiguousarray(xs[c * FPC:(c + 1) * FPC])
        in_maps.append(m)
    t2 = _t.time()
    res = run_bass_kernel_spmd(nc_vgg, in_maps, core_ids=list(range(NCORES)))
    t3 = _t.time()
    feats = np.concatenate([res.results[c]["feats"] for c in range(NCORES)], axis=0)

    dmap = _prep_decode(feats, previous_state, att_w, gru_params, pre_w, pre_b,
                        norm2_w, norm2_b, cls_w, cls_b, norm3_w, norm3_b)
    res2 = run_bass_kernel_spmd(nc_dec, [dmap], core_ids=[0])
    t4 = _t.time()
    import sys as _sys
    print(f"[kernel] wprep {t1-t0:.2f}s im2col {t2-t1:.2f}s vgg {t3-t2:.2f}s "
          f"decode {t4-t3:.2f}s", file=_sys.stderr, flush=True)
    o = np.asarray(res2.results[0]["o"], np.float32).reshape(1, 11)
    hf = np.asarray(res2.results[0]["hf"], np.float32).reshape(2, 1, PH)
    return (o, hf)


# revision 27
# speedup vs baseline: 1.1418x; 1.1418x over previous
"""Trainium2 kernel for nn_Attention_15092515078765.

Sharding: data-parallel over the 32 frames (4 per core) for the VGG13
feature extractor; the sequential attention+GRU decode runs on core 0.

Math shortcut (validated against the reference): the additive-attention
logits are cat([out_t, f]) @ att_w + att_b, and the out_t-dependent term is
identical for every sequence position, so the softmax is invariant to it.
Hence the attention weights -- and the GRU layer-1 input context -- are
constant across all 32 decode steps.
"""

import copy
import json

import numpy as np

import concourse.bass as bass
import concourse.tile as tile
from concourse import mybir
from concourse.bass_utils import run_bass_kernel_spmd

F32 = mybir.dt.float32
AF = mybir.ActivationFunctionType
ALU = mybir.AluOpType

NCORES = 8
FPC = 4            # frames per core
L = 32             # sequence length
PH = 48
EPS = 1e-5

# layer geometry: (cin, cout, H, W); spatial size is at conv input == output
VGG = [
    (3, 64, 160, 96), (64, 64, 160, 96),
    (64, 128, 80, 48), (128, 128, 80, 48),
    (128, 256, 40, 24), (256, 256, 40, 24), (256, 256, 40, 24),
    (256, 512, 20, 12), (512, 512, 20, 12), (512, 512, 20, 12),
    (512, 512, 10, 6), (512, 512, 10, 6), (512, 512, 10, 6),
]

# bias column packing: (li, mb) -> column in the biases array
BIAS_COL = {}
_c = 0
for _li, (_ci, _co, _h, _w) in enumerate(VGG):
    for _mb in range(max(1, _co // 128)):
        BIAS_COL[(_li, _mb)] = _c
        _c += 1
N_BIAS_COL = _c


# --------------------------------------------------------------------------
# TileContext with a walrus-compatible exit: this toolchain's walrus rejects
# >1 sem-wait per instruction, and we avoid the EVSEM butterfly barrier.
# --------------------------------------------------------------------------
class TC(tile.TileContext):
    def _drain_and_barrier(self, tick_clock, wait_clock):
        nc = self.nc
        vc = tick_clock.global_clock
        assert self.sems is not None
        alloc = self.sems.allocated()
        for proc_idx, sem in sorted(alloc.items()):
            tick = vc[proc_idx]
            if tick > 0:
                inc = 16 if 11 <= proc_idx <= 26 else 1
                nc.gpsimd.wait_ge(sem, tick * inc)
        # plain-semaphore all-engine barrier (no EVSEM butterfly, no multi-wait
        # instructions): gpsimd -> done -> every engine acks -> gpsimd clears.
        done = nc.alloc_semaphore("exit_done")
        ack = nc.alloc_semaphore("exit_ack")
        nc.gpsimd.nop().then_inc(done, 1)
        for eng in (nc.tensor, nc.vector, nc.scalar, nc.sync):
            eng.wait_ge(done, 1)
            eng.nop().then_inc(ack, 1)
        nc.gpsimd.wait_ge(ack, 4)
        popped = nc._tile_sem_poison_stack.pop()
        assert popped is self._sem_poison
        nc.clear_and_free_semaphores(list(alloc.values()))
        nc.gpsimd.sem_clear(done)
        nc.gpsimd.sem_clear(ack)

    def _queue_alloc(self, pool):
        # best-fit (not first-fit) gap selection to curb fragmentation
        import concourse.bass as _b
        from concourse._compat import round_up_to_multiple, exact_div
        align = 32
        if self._queue_ring is None:
            ring_base = round_up_to_multiple(self.nc.sbuf_base, align)
            ring_end = self.nc.sbuf_top
            self.nc.sbuf_base = ring_end
            self._queue_ring = (ring_base, ring_end, ring_base)
        ring_base, ring_end, head = self._queue_ring
        k = exact_div(pool.size, self.nc.NUM_PARTITIONS)
        live = sorted(
            la
            for side in ("left", "right")
            for lv in self.pool_stacks[(_b.MemorySpace.SBUF, side)]
            if (la := lv._ring_addr) is not None
        )
        gaps = []
        cur = ring_base
        for lb, le in live:
            if lb > cur:
                gaps.append((cur, lb))
            cur = max(cur, le)
        if ring_end > cur:
            gaps.append((cur, ring_end))
        # static plan for the VGG kernel's pools (lifetime-verified);
        # unknown pools fall back to two-ended placement.
        PLACE = {
            "const": 16512, "tmp": 20160,
            "pin": 36000, "a1": 99552, "a2": 163072, "w01": 178432,
            "p1": 36000, "a3": 101632, "w2": 167264, "w3": 171904,
            "w4": 176544, "a4": 36000, "a4q": 101632, "w5": 119136,
            "a5": 36000, "w6": 137600, "a6": 70976, "w7": 156064,
            "a7": 36000, "a7q": 66752, "w8a": 81888, "a8": 36000,
            "w8b": 118784, "w9a": 155680, "a9": 55744, "w9b": 81888,
            "w10a": 118784, "a10": 36000, "w10b": 81888, "a10q": 51392,
            "w11a": 155680, "a11": 57568, "w11b": 81888, "w12a": 118784,
            "a12": 63744, "w12b": 81888, "a13": 36000, "xf": 39872,
            "stat": 43744,
        }
        if pool.name in PLACE:
            base_addr = PLACE[pool.name]
            end_addr = base_addr + k
            assert end_addr <= ring_end, (pool.name, k, base_addr, ring_end)
            for lb, le in live:
                assert not (base_addr < le and lb < end_addr), (
                    f"plan conflict: {pool.name} [{base_addr},{end_addr}) vs "
                    f"live [{lb},{le})")
            self._queue_ring = (ring_base, ring_end, end_addr)
            pool._ring_addr = (base_addr, end_addr)
            return base_addr, end_addr
        fitting = []
        for gb, ge in gaps:
            gb = round_up_to_multiple(gb, align)
            if ge - gb >= k:
                fitting.append((gb, ge))
        if not fitting:
            raise ValueError(
                f"queue ring full: {pool.name=} ({k}B/part) no gap; "
                f"gaps={[(g[0], g[1] - g[0]) for g in gaps]}")
        if k > 24 * 1024:
            gb, ge = fitting[-1]
            base_addr = (ge - k) // align * align
            assert base_addr >= gb
        else:
            gb, ge = fitting[0]
            base_addr = gb
        end_addr = base_addr + k
        self._queue_ring = (ring_base, ring_end, end_addr)
        pool._ring_addr = (base_addr, end_addr)
        return base_addr, end_addr

    def _process_pool_release(self, pool, inst):
        # allow non-LIFO release for queue-mode pools (ring allocator): the
        # stack is only bookkeeping there; remove by identity instead.
        stack_key = (pool.space, pool.side)
        stack = self.pool_stacks[stack_key]
        if self._is_queue_pool(pool) and (not stack or stack[-1] is not pool):
            for i in range(len(stack) - 1, -1, -1):
                if stack[i] is pool:
                    del stack[i]
                    break
            else:
                raise AssertionError(f"pool {pool.name} not on stack")
            assert pool._ring_addr is not None
            base_addr, end_addr = pool._ring_addr
            self.released_zones[pool.space].append((base_addr, end_addr, inst))
            return
        super()._process_pool_release(pool, inst)


class PoolChain:
    """Open/close tile pools in arbitrary (non-LIFO) order (queue alloc)."""

    def __init__(self, tc):
        self.tc = tc
        self.cms = {}

    def open(self, name, **kw):
        cm = self.tc.tile_pool(name=name, **kw)
        pool = cm.__enter__()
        self.cms[name] = cm
        return pool

    def close(self, name):
        self.cms.pop(name).__exit__(None, None, None)

    def close_all(self):
        for name in reversed(list(self.cms)):
            self.close(name)


def _zero_borders(nc, t, nb, H, W):
    """t: [P, nb, FPC, H+2, W+2] padded activation; zero the 1-px borders."""
    for b in range(nb):
        for f in range(FPC):
            v = t[:, b, f]
            nc.gpsimd.memset(v[:, 0:1, :], 0.0)
            nc.gpsimd.memset(v[:, H + 1:H + 2, :], 0.0)
            nc.gpsimd.memset(v[:, 1:H + 1, 0:1], 0.0)
            nc.gpsimd.memset(v[:, 1:H + 1, W + 1:W + 2], 0.0)


def fix_multi_waits(nc):
    """This walrus pin rejects >1 sem-wait per instruction: hoist extra
    waits onto single-wait EventSemaphore instructions just before."""
    nc.finalize()
    js = json.loads(mybir.module_to_json_string(nc.m))
    template = None
    for f in js["functions"]:
        for bb in f["blocks"]:
            for inst in bb["instructions"]:
                if inst.get("opcode") == "EventSemaphore":
                    template = copy.deepcopy(inst)
                    break
            if template:
                break
        if template:
            break
    assert template is not None, "no EventSemaphore template found"
    n_new = [0]
    n_split = 0
    for f in js["functions"]:
        for bb in f["blocks"]:
            out = []
            for inst in bb["instructions"]:
                si = inst.get("sync_info")
                waits = (si or {}).get("on_wait") or []
                if len(waits) > 1:
                    n_split += 1
                    for w in waits[:-1]:
                        n_new[0] += 1
                        t = copy.deepcopy(template)
                        t["name"] = f"I-mw{n_new[0]}"
                        t["engine"] = inst["engine"]
                        t["sync_info"] = {"on_update": [], "on_wait": [w]}
                        out.append(t)
                    si["on_wait"] = [waits[-1]]
                out.append(inst)
            bb["instructions"] = out
    nc.m = mybir.module_from_json_string(json.dumps(js))
    return n_split


def build_vgg_nc():
    """Per-core program: xs [4,3,160,96] -> feats [4,48]."""
    nc = bass.Bass()
    xs = nc.dram_tensor("xs", [FPC, 27, 162, 98], F32, kind="ExternalInput")
    BF16 = mybir.dt.bfloat16
    wd = {0: nc.dram_tensor("w0", [27, 64], F32, kind="ExternalInput")}
    for li in range(1, 13):
        cin, cout, _, _ = VGG[li]
        cinb, coutb = max(1, cin // 128), max(1, cout // 128)
        cinP, coutP = min(cin, 128), min(cout, 128)
        wd[li] = nc.dram_tensor(f"w{li}", [cinP, coutb, cinb, 3, 3, coutP],
                                F32, kind="ExternalInput")
    biases = nc.dram_tensor("biases", [128, N_BIAS_COL], F32, kind="ExternalInput")
    n0w = nc.dram_tensor("n0w", [128, 4, 15], F32, kind="ExternalInput")
    n0b = nc.dram_tensor("n0b", [128, 4, 15], F32, kind="ExternalInput")
    cnnw = nc.dram_tensor("cnnw", [128, 4, 3, 3, 16], F32, kind="ExternalInput")
    cnnb = nc.dram_tensor("cnnb", [16, 1], F32, kind="ExternalInput")
    n1w = nc.dram_tensor("n1w", [16, 3], F32, kind="ExternalInput")
    n1b = nc.dram_tensor("n1b", [16, 3], F32, kind="ExternalInput")
    feats_out = nc.dram_tensor("feats", [FPC, PH], F32, kind="ExternalOutput")
    p1scratch = nc.dram_tensor("p1scratch", [64, FPC, 80, 48], F32)

    with TC(nc, pool_alloc_mode="queue") as tc:
        ch = PoolChain(tc)
        psum = ch.open("ps", bufs=6, space="PSUM")
        tmpp = ch.open("tmp", bufs=1)
        const = ch.open("const", bufs=1)

        bias_sb = const.tile([128, N_BIAS_COL], F32)
        nc.gpsimd.dma_start(out=bias_sb[:, :], in_=biases[:, :])
        n0w_sb = const.tile([128, 4, 15], F32)
        nc.gpsimd.dma_start(out=n0w_sb[:, :, :], in_=n0w[:, :, :])
        n0b_sb = const.tile([128, 4, 15], F32)
        nc.gpsimd.dma_start(out=n0b_sb[:, :, :], in_=n0b[:, :, :])
        cnnw_sb = const.tile([128, 4, 3, 3, 16], F32)
        nc.gpsimd.dma_start(out=cnnw_sb[:, :, :, :, :], in_=cnnw[:, :, :, :, :])
        cnnb_sb = const.tile([16, 1], F32)
        nc.gpsimd.dma_start(out=cnnb_sb[:, :], in_=cnnb[:, :])
        n1w_sb = const.tile([16, 3], F32)
        nc.gpsimd.dma_start(out=n1w_sb[:, :], in_=n1w[:, :])
        n1b_sb = const.tile([16, 3], F32)
        nc.gpsimd.dma_start(out=n1b_sb[:, :], in_=n1b[:, :])
        eps_sb = const.tile([1, 1], F32)
        nc.vector.memset(eps_sb[:, :], EPS)
        ones_row = const.tile([1, 128], F32)
        nc.vector.memset(ones_row[:, :], 1.0)
        ones_col = const.tile([128, 1], F32)
        nc.vector.memset(ones_col[:, :], 1.0)

        def bias_ap(li, mb, coutP):
            col = BIAS_COL[(li, mb)]
            return bias_sb[0:coutP, col:col + 1]

        def load_w(li):
            cin, cout, _, _ = VGG[li]
            cinb, coutb = max(1, cin // 128), max(1, cout // 128)
            cinP, coutP = min(cin, 128), min(cout, 128)
            p = ch.open(f"w{li}", bufs=1)
            t = p.tile([cinP, coutb, cinb, 3, 3, coutP], F32)
            nc.gpsimd.dma_start(out=t[:, :, :, :, :, :], in_=wd[li][:, :, :, :, :, :])
            return t

        def load_w_half(li, half):
            cin, cout, _, _ = VGG[li]
            cinb = max(1, cin // 128)
            cinP = min(cin, 128)
            p = ch.open(f"w{li}{'ab'[half]}", bufs=1)
            t = p.tile([cinP, 2, cinb, 3, 3, 128], F32)
            nc.gpsimd.dma_start(
                out=t[:, :, :, :, :, :],
                in_=wd[li][:, 2 * half:2 * half + 2, :, :, :, :])
            return t

        def close_w_half(li):
            ch.close(f"w{li}a"); ch.close(f"w{li}b")

        # ================= stage A: conv1 + conv2 + pool1, per frame ========
        w01p = ch.open("w01", bufs=1)
        w0_sb = w01p.tile([27, 64], F32)
        nc.gpsimd.dma_start(out=w0_sb[:, :], in_=wd[0][:, :])
        w1_sb = w01p.tile([64, 1, 1, 3, 3, 64], F32)
        nc.gpsimd.dma_start(out=w1_sb[:, :, :, :, :, :], in_=wd[1][:, :, :, :, :, :])

        pinp = ch.open("pin", bufs=1)
        pin = pinp.tile([27, 162, 98], F32)
        a1p = ch.open("a1", bufs=1)
        a1 = a1p.tile([64, 162, 98], F32)
        a2p = ch.open("a2", bufs=1)
        a2 = a2p.tile([64, 40, 96], F32)

        nc.gpsimd.memset(a1[:, 0:1, :], 0.0)
        nc.gpsimd.memset(a1[:, 161:162, :], 0.0)
        nc.gpsimd.memset(a1[:, 1:161, 0:1], 0.0)
        nc.gpsimd.memset(a1[:, 1:161, 97:98], 0.0)

        for f in range(FPC):
            nc.gpsimd.dma_start(out=pin[:, :, :], in_=xs[f, :, :, :])
            # conv1: 32 tiles of 5 rows, K=27 via im2col
            for i in range(32):
                ps = psum.tile([64, 5, 96], F32, tag="ps")
                nc.tensor.matmul(ps[:, :, :], w0_sb[:, :],
                                 pin[:, 1 + 5 * i:6 + 5 * i, 1:97],
                                 start=True, stop=True)
                nc.scalar.activation(out=a1[:, 1 + 5 * i:6 + 5 * i, 1:97],
                                     in_=ps[:, :, :], func=AF.Relu,
                                     bias=bias_ap(0, 0, 64), scale=1.0)
            # conv2 (9 accumulated taps) in 4 chunks of 40 rows, + pool1
            for c4 in range(4):
                for i in range(8):
                    r0 = c4 * 40 + i * 5
                    ps = psum.tile([64, 5, 96], F32, tag="ps")
                    for t9 in range(9):
                        ti, tj = t9 // 3, t9 % 3
                        nc.tensor.matmul(ps[:, :, :], w1_sb[:, 0, 0, ti, tj, :],
                                         a1[:, r0 + ti:r0 + ti + 5, tj:tj + 96],
                                         start=(t9 == 0), stop=(t9 == 8))
                    nc.scalar.activation(out=a2[:, 5 * i:5 * i + 5, :],
                                         in_=ps[:, :, :], func=AF.Relu,
                                         bias=bias_ap(1, 0, 64), scale=1.0)
                for hh in range(2):
                    tmp = tmpp.tile([64, 20, 48], F32, tag="tmpA")
                    v = a2[:, 20 * hh:20 * hh + 20, :].rearrange(
                        "p h (w two) -> p h w two", two=2)
                    nc.vector.tensor_max(tmp[:, :, :], v[:, :, :, 0], v[:, :, :, 1])
                    pout = tmpp.tile([64, 10, 48], F32, tag="poolout")
                    u = tmp.rearrange("p (h two) w -> p h two w", two=2)
                    nc.vector.tensor_max(pout[:, :, :], u[:, :, 0, :], u[:, :, 1, :])
                    nc.gpsimd.dma_start(
                        out=p1scratch[:, f, 20 * c4 + 10 * hh:20 * c4 + 10 * hh + 10, :],
                        in_=pout[:, :, :])

        ch.close("pin"); ch.close("a1"); ch.close("a2"); ch.close("w01")
        p1pool = ch.open("p1", bufs=1)
        p1 = p1pool.tile([64, 1, FPC, 82, 50], F32)
        _zero_borders(nc, p1, 1, 80, 48)
        for f in range(FPC):
            nc.gpsimd.dma_start(out=p1[:, 0, f, 1:81, 1:49],
                                in_=p1scratch[:, f, :, :])

        # ================= stage B: conv3..conv13, 4 frames batched =========
        def conv_b(src, dst, li, w_sb, rows_per_tile, dst_padded):
            cin, cout, H, W = VGG[li]
            cinb, coutb = max(1, cin // 128), max(1, cout // 128)
            cinP, coutP = min(cin, 128), min(cout, 128)
            for mb in range(coutb):
                if isinstance(w_sb, tuple):
                    wt, mbl = w_sb[mb // 2], mb % 2
                else:
                    wt, mbl = w_sb, mb
                for r0 in range(0, H, rows_per_tile):
                    rows = min(rows_per_tile, H - r0)
                    ps = psum.tile([coutP, FPC, rows, W], F32, tag="ps")
                    n = 0
                    for cb in range(cinb):
                        for t9 in range(9):
                            ti, tj = t9 // 3, t9 % 3
                            nc.tensor.matmul(
                                ps[:, :, :, :], wt[:, mbl, cb, ti, tj, :],
                                src[:, cb, :, r0 + ti:r0 + ti + rows, tj:tj + W],
                                start=(n == 0), stop=(n == cinb * 9 - 1))
                            n += 1
                    if dst_padded:
                        out = dst[:, mb, :, 1 + r0:1 + r0 + rows, 1:W + 1]
                    else:
                        out = dst[:, mb, :, r0:r0 + rows, :]
                    nc.scalar.activation(out=out, in_=ps[:, :, :, :], func=AF.Relu,
                                         bias=bias_ap(li, mb, coutP), scale=1.0)

        def maxpool_b(src, dst, nb, H, W):
            Ho, Wo = H // 2, W // 2
            for b in range(nb):
                for f in range(FPC):
                    tmp = tmpp.tile([128, H, Wo], F32, tag="tmpB")
                    v = src[:, b, f].rearrange("p h (w two) -> p h w two", two=2)
                    nc.vector.tensor_max(tmp[:, :, :], v[:, :, :, 0], v[:, :, :, 1])
                    u = tmp.rearrange("p (h two) w -> p h two w", two=2)
                    nc.vector.tensor_max(dst[:, b, f, 1:Ho + 1, 1:Wo + 1],
                                         u[:, :, 0, :], u[:, :, 1, :])

        def new_act(name, nb, H, W, padded=True):
            p = ch.open(name, bufs=1)
            if padded:
                t = p.tile([128, nb, FPC, H + 2, W + 2], F32)
                _zero_borders(nc, t, nb, H, W)
            else:
                t = p.tile([128, nb, FPC, H, W], F32)
            return t

        w2 = load_w(2); w3 = load_w(3)
        a3 = new_act("a3", 1, 80, 48)
        conv_b(p1, a3, 2, w2, 2, True)
        ch.close("p1"); ch.close("w2")
        w4 = load_w(4)
        a4 = new_act("a4", 1, 80, 48, padded=False)
        conv_b(a3, a4, 3, w3, 2, False)
        ch.close("a3"); ch.close("w3")
        a4q = new_act("a4q", 1, 40, 24)
        maxpool_b(a4, a4q, 1, 80, 48)
        ch.close("a4")
        w5 = load_w(5)
        a5 = new_act("a5", 2, 40, 24)
        conv_b(a4q, a5, 4, w4, 5, True)
        ch.close("a4q"); ch.close("w4")
        w6 = load_w(6)
        a6 = new_act("a6", 2, 40, 24)
        conv_b(a5, a6, 5, w5, 5, True)
        ch.close("a5"); ch.close("w5")
        w7 = load_w(7)
        a7 = new_act("a7", 2, 40, 24, padded=False)
        conv_b(a6, a7, 6, w6, 5, False)
        ch.close("a6"); ch.close("w6")
        a7q = new_act("a7q", 2, 20, 12)
        maxpool_b(a7, a7q, 2, 40, 24)
        ch.close("a7")
        w8a = load_w_half(8, 0)
        a8 = new_act("a8", 4, 20, 12)
        conv_b(a7q, a8, 7, w7, 9, True)
        w8b = load_w_half(8, 1)
        ch.close("a7q"); ch.close("w7")
        w9a = load_w_half(9, 0)
        a9 = new_act("a9", 4, 20, 12)
        conv_b(a8, a9, 8, (w8a, w8b), 9, True)
        ch.close("a8"); close_w_half(8)
        w9b = load_w_half(9, 1)
        w10a = load_w_half(10, 0)
        a10 = new_act("a10", 4, 20, 12, padded=False)
        conv_b(a9, a10, 9, (w9a, w9b), 9, False)
        ch.close("a9"); close_w_half(9)
        w10b = load_w_half(10, 1)
        a10q = new_act("a10q", 4, 10, 6)
        maxpool_b(a10, a10q, 4, 20, 12)
        ch.close("a10")
        w11a = load_w_half(11, 0)
        a11 = new_act("a11", 4, 10, 6)
        conv_b(a10q, a11, 10, (w10a, w10b), 10, True)
        ch.close("a10q"); close_w_half(10)
        w11b = load_w_half(11, 1)
        w12a = load_w_half(12, 0)
        a12 = new_act("a12", 4, 10, 6)
        conv_b(a11, a12, 11, (w11a, w11b), 10, True)
        ch.close("a11"); close_w_half(11)
        w12b = load_w_half(12, 1)
        a13 = new_act("a13", 4, 10, 6, padded=False)
        conv_b(a12, a13, 12, (w12a, w12b), 10, False)
        ch.close("a12"); close_w_half(12)

        # pool5 -> xf [128, 4, FPC, 5, 3]
        xfp = ch.open("xf", bufs=1)
        xf = xfp.tile([128, 4, FPC, 5, 3], F32)
        for b in range(4):
            for f in range(FPC):
                tmp = tmpp.tile([128, 10, 3], F32, tag="tmpB")
                v = a13[:, b, f].rearrange("p h (w two) -> p h w two", two=2)
                nc.vector.tensor_max(tmp[:, :, :], v[:, :, :, 0], v[:, :, :, 1])
                u = tmp.rearrange("p (h two) w -> p h two w", two=2)
                nc.vector.tensor_max(xf[:, b, f, :, :], u[:, :, 0, :], u[:, :, 1, :])
        ch.close("a13")

        # ---------------- norm0 (LN over 7680 per frame) --------------------
        statp = ch.open("stat", bufs=1)
        sq = statp.tile([128, 4, FPC, 15], F32)
        xfv = xf.rearrange("p b f h w -> p b f (h w)")
        nc.vector.tensor_mul(sq[:, :, :, :], xfv, xfv)
        colsum = statp.tile([1, 4, FPC, 15], F32)
        colsq = statp.tile([1, 4, FPC, 15], F32)
        ps_cs = psum.tile([1, 4, FPC, 15], F32, tag="ps")
        nc.tensor.matmul(ps_cs[:, :, :, :], ones_col[:, :], xfv,
                         start=True, stop=True)
        nc.vector.tensor_copy(colsum[:, :, :, :], ps_cs[:, :, :, :])
        ps_cq = psum.tile([1, 4, FPC, 15], F32, tag="ps")
        nc.tensor.matmul(ps_cq[:, :, :, :], ones_col[:, :], sq[:, :, :, :],
                         start=True, stop=True)
        nc.vector.tensor_copy(colsq[:, :, :, :], ps_cq[:, :, :, :])
        fsum = statp.tile([1, FPC, 2], F32)
        for f in range(FPC):
            nc.vector.tensor_reduce(fsum[:, f, 0:1], colsum[:, :, f, :],
                                    axis=mybir.AxisListType.XY, op=ALU.add)
            nc.vector.tensor_reduce(fsum[:, f, 1:2], colsq[:, :, f, :],
                                    axis=mybir.AxisListType.XY, op=ALU.add)
        mv = statp.tile([1, FPC, 3], F32)          # mu, rstd, -mu
        nc.scalar.mul(mv[:, :, 0:1], fsum[:, :, 0:1], 1.0 / 7680.0)
        musq = statp.tile([1, FPC, 1], F32)
        nc.vector.tensor_mul(musq[:, :, :], mv[:, :, 0:1], mv[:, :, 0:1])
        var = statp.tile([1, FPC, 1], F32)
        nc.vector.scalar_tensor_tensor(var[:, :, :], fsum[:, :, 1:2], 1.0 / 7680.0,
                                       musq[:, :, :], op0=ALU.mult,
                                       op1=ALU.subtract)
        sd = statp.tile([1, FPC, 1], F32)
        nc.scalar.activation(sd[:, :, :], var[:, :, :], AF.Sqrt, bias=eps_sb[:, 0:1], scale=1.0)
        nc.vector.reciprocal(mv[:, :, 1:2], sd[:, :, :])
        nc.scalar.mul(mv[:, :, 2:3], mv[:, :, 0:1], -1.0)
        bc = statp.tile([128, FPC, 2], F32)        # [...,0]=rstd, [...,1]=-mu
        ps_b = psum.tile([128, FPC, 2], F32, tag="ps")
        nc.tensor.matmul(ps_b[:, :, :], ones_row[:, :], mv[:, :, 1:3],
                         start=True, stop=True)
        nc.vector.tensor_copy(bc[:, :, :], ps_b[:, :, :])
        xn = statp.tile([128, 4, FPC, 15], F32)
        for f in range(FPC):
            nc.vector.tensor_scalar(out=xn[:, :, f, :], in0=xfv[:, :, f, :],
                                    scalar1=bc[:, f, 1:2], scalar2=bc[:, f, 0:1],
                                    op0=ALU.add, op1=ALU.mult)
            nc.vector.tensor_mul(xn[:, :, f, :], xn[:, :, f, :], n0w_sb[:, :, :])
            nc.vector.tensor_add(xn[:, :, f, :], xn[:, :, f, :], n0b_sb[:, :, :])

        # ---------------- cnn 512->16 (3x3 valid on 5x3 -> 3x1), leaky ------
        xn5 = xn.rearrange("p b f (h w) -> p b f h w", w=3)
        psc = psum.tile([16, FPC, 3], F32, tag="ps")
        n = 0
        for cb in range(4):
            for ti in range(3):
                for tj in range(3):
                    nc.tensor.matmul(psc[:, :, :], cnnw_sb[:, cb, ti, tj, :],
                                     xn5[:, cb, :, ti:ti + 3, tj:tj + 1],
                                     start=(n == 0), stop=(n == 35))
                    n += 1
        fpre = statp.tile([16, FPC, 3], F32)
        xb = statp.tile([16, FPC, 3], F32)
        nc.vector.tensor_scalar_add(xb[:, :, :], in0=psc[:, :, :],
                                    scalar1=cnnb_sb[:, 0:1])
        xmn = statp.tile([16, FPC, 3], F32)
        nc.vector.tensor_scalar_min(xmn[:, :, :], in0=xb[:, :, :], scalar1=0.0)
        xmx = statp.tile([16, FPC, 3], F32)
        nc.vector.tensor_scalar_max(xmx[:, :, :], in0=xb[:, :, :], scalar1=0.0)
        nc.vector.scalar_tensor_tensor(fpre[:, :, :], xmn[:, :, :], 0.01,
                                       xmx[:, :, :], op0=ALU.mult, op1=ALU.add)

        # ---------------- norm1 (LN over 48 per frame) ----------------------
        sq1 = statp.tile([16, FPC, 3], F32)
        nc.vector.tensor_mul(sq1[:, :, :], fpre[:, :, :], fpre[:, :, :])
        c1 = statp.tile([1, FPC, 3], F32)
        c2 = statp.tile([1, FPC, 3], F32)
        ps_c1 = psum.tile([1, FPC, 3], F32, tag="ps")
        nc.tensor.matmul(ps_c1[:, :, :], ones_col[0:16, :], fpre[:, :, :],
                         start=True, stop=True)
        nc.vector.tensor_copy(c1[:, :, :], ps_c1[:, :, :])
        ps_c2 = psum.tile([1, FPC, 3], F32, tag="ps")
        nc.tensor.matmul(ps_c2[:, :, :], ones_col[0:16, :], sq1[:, :, :],
                         start=True, stop=True)
        nc.vector.tensor_copy(c2[:, :, :], ps_c2[:, :, :])
        f1 = statp.tile([1, FPC, 2], F32)
        for f in range(FPC):
            nc.vector.tensor_reduce(f1[:, f, 0:1], c1[:, f, :],
                                    axis=mybir.AxisListType.X, op=ALU.add)
            nc.vector.tensor_reduce(f1[:, f, 1:2], c2[:, f, :],
                                    axis=mybir.AxisListType.X, op=ALU.add)
        mv1 = statp.tile([1, FPC, 3], F32)
        nc.scalar.mul(mv1[:, :, 0:1], f1[:, :, 0:1], 1.0 / 48.0)
        musq1 = statp.tile([1, FPC, 1], F32)
        nc.vector.tensor_mul(musq1[:, :, :], mv1[:, :, 0:1], mv1[:, :, 0:1])
        var1 = statp.tile([1, FPC, 1], F32)
        nc.vector.scalar_tensor_tensor(var1[:, :, :], f1[:, :, 1:2], 1.0 / 48.0,
                                       musq1[:, :, :], op0=ALU.mult,
                                       op1=ALU.subtract)
        sd1 = statp.tile([1, FPC, 1], F32)
        nc.scalar.activation(sd1[:, :, :], var1[:, :, :], AF.Sqrt,
                             bias=eps_sb[:, 0:1], scale=1.0)
        nc.vector.reciprocal(mv1[:, :, 1:2], sd1[:, :, :])
        nc.scalar.mul(mv1[:, :, 2:3], mv1[:, :, 0:1], -1.0)
        bc1 = statp.tile([16, FPC, 2], F32)        # [...,0]=rstd, [...,1]=-mu
        ps_b1 = psum.tile([16, FPC, 2], F32, tag="ps")
        nc.tensor.matmul(ps_b1[:, :, :], ones_row[0:1, 0:16], mv1[:, :, 1:3],
                         start=True, stop=True)
        nc.vector.tensor_copy(bc1[:, :, :], ps_b1[:, :, :])
        feats_sb = statp.tile([16, FPC, 3], F32)
        for f in range(FPC):
            nc.vector.tensor_scalar(out=feats_sb[:, f, :], in0=fpre[:, f, :],
                                    scalar1=bc1[:, f, 1:2], scalar2=bc1[:, f, 0:1],
                                    op0=ALU.add, op1=ALU.mult)
            nc.vector.tensor_mul(feats_sb[:, f, :], feats_sb[:, f, :], n1w_sb[:, :])
            nc.vector.tensor_add(feats_sb[:, f, :], feats_sb[:, f, :], n1b_sb[:, :])
        nc.gpsimd.dma_start(
            out=feats_out[:, :].rearrange("f (c p) -> c f p", p=3),
            in_=feats_sb[:, :, :])
        ch.close_all()
    return nc


def build_decode_nc():
    """Core-0 program: feats [32,48] + recurrent weights -> o [1,11], hf [2,1,48]."""
    nc = bass.Bass()
    f_d = nc.dram_tensor("f", [L, PH], F32, kind="ExternalInput")
    fT_d = nc.dram_tensor("fT", [PH, L], F32, kind="ExternalInput")
    w2_d = nc.dram_tensor("w2col", [PH, 1], F32, kind="ExternalInput")
    wih1_d = nc.dram_tensor("wih1T", [PH, 3 * PH], F32, kind="ExternalInput")
    whh1_d = nc.dram_tensor("whh1T", [PH, 3 * PH], F32, kind="ExternalInput")
    wih2_d = nc.dram_tensor("wih2T", [PH, 3 * PH], F32, kind="ExternalInput")
    whh2_d = nc.dram_tensor("whh2T", [PH, 3 * PH], F32, kind="ExternalInput")
    dc_d = nc.dram_tensor("dc", [PH, 8], F32, kind="ExternalInput")
    h0_d = nc.dram_tensor("h0", [PH, 2], F32, kind="ExternalInput")
    prew_d = nc.dram_tensor("prewT", [PH, 16], F32, kind="ExternalInput")
    clsw_d = nc.dram_tensor("clswT", [16, 11], F32, kind="ExternalInput")
    hc_d = nc.dram_tensor("hc", [16, 6], F32, kind="ExternalInput")
    o_out = nc.dram_tensor("o", [1, 11], F32, kind="ExternalOutput")
    hf_out = nc.dram_tensor("hf", [2, PH], F32, kind="ExternalOutput")

    with TC(nc, pool_alloc_mode="queue") as tc:
        ch = PoolChain(tc)
        ps = ch.open("ps", bufs=8, space="PSUM")
        sb = ch.open("sb", bufs=1)
        st = ch.open("st", bufs=2)

        f_sb = sb.tile([L, PH], F32)
        nc.gpsimd.dma_start(out=f_sb[:, :], in_=f_d[:, :])
        fT_sb = sb.tile([PH, L], F32)
        nc.gpsimd.dma_start(out=fT_sb[:, :], in_=fT_d[:, :])
        w2_sb = sb.tile([PH, 1], F32)
        nc.gpsimd.dma_start(out=w2_sb[:, :], in_=w2_d[:, :])
        wih1 = sb.tile([PH, 3 * PH], F32)
        nc.gpsimd.dma_start(out=wih1[:, :], in_=wih1_d[:, :])
        whh1 = sb.tile([PH, 3 * PH], F32)
        nc.gpsimd.dma_start(out=whh1[:, :], in_=whh1_d[:, :])
        wih2 = sb.tile([PH, 3 * PH], F32)
        nc.gpsimd.dma_start(out=wih2[:, :], in_=wih2_d[:, :])
        whh2 = sb.tile([PH, 3 * PH], F32)
        nc.gpsimd.dma_start(out=whh2[:, :], in_=whh2_d[:, :])
        dc = sb.tile([PH, 8], F32)
        nc.gpsimd.dma_start(out=dc[:, :], in_=dc_d[:, :])
        h0 = sb.tile([PH, 2], F32)
        nc.gpsimd.dma_start(out=h0[:, :], in_=h0_d[:, :])
        prew = sb.tile([PH, 16], F32)
        nc.gpsimd.dma_start(out=prew[:, :], in_=prew_d[:, :])
        clsw = sb.tile([16, 11], F32)
        nc.gpsimd.dma_start(out=clsw[:, :], in_=clsw_d[:, :])
        hc = sb.tile([16, 6], F32)
        nc.gpsimd.dma_start(out=hc[:, :], in_=hc_d[:, :])
        eps16 = sb.tile([16, 1], F32)
        nc.vector.memset(eps16[:, :], EPS)
        ones_c = sb.tile([16, 1], F32)
        nc.vector.memset(ones_c[:, :], 1.0)
        ones_r = sb.tile([1, 16], F32)
        nc.vector.memset(ones_r[:, :], 1.0)

        # attention (constant across steps): a = softmax(f @ w2); ctx = a @ f
        ps_l = ps.tile([1, L], F32, tag="pp")
        nc.tensor.matmul(ps_l[:, :], w2_sb[:, :], fT_sb[:, :], start=True, stop=True)
        mx = sb.tile([1, 1], F32)
        nc.vector.reduce_max(mx[:, :], ps_l[:, :], axis=mybir.AxisListType.X)
        negm = sb.tile([1, 1], F32)
        nc.scalar.mul(negm[:, :], mx[:, :], -1.0)
        e = sb.tile([1, L], F32)
        esum = sb.tile([1, 1], F32)
        nc.scalar.activation(e[:, :], ps_l[:, :], AF.Exp, bias=negm[:, 0:1],
                             scale=1.0, accum_out=esum[:, :])
        rs = sb.tile([1, 1], F32)
        nc.vector.reciprocal(rs[:, :], esum[:, :])
        a_row = sb.tile([1, L], F32)
        nc.vector.tensor_scalar_mul(a_row[:, :], in0=e[:, :], scalar1=rs[:, 0:1])
        a_col = sb.tile([L, 1], F32)
        nc.gpsimd.dma_start(out=a_col[:, :], in_=a_row[:, :])
        ps_c = ps.tile([PH, 1], F32, tag="pp")
        nc.tensor.matmul(ps_c[:, :], f_sb[:, :], a_col[:, :], start=True, stop=True)
        ctx = sb.tile([PH, 1], F32)
        nc.vector.tensor_copy(ctx[:, :], ps_c[:, :])

        # gi1 = W_ih1 @ ctx (+ gate-combined biases), constant across steps
        gib = sb.tile([PH, 3], F32)      # cols: r-bias, z-bias, n-part
        for g in range(3):
            pg = ps.tile([PH, 1], F32, tag="pp")
            nc.tensor.matmul(pg[:, :], wih1[:, 48 * g:48 * g + 48], ctx[:, :],
                             start=True, stop=True)
            nc.vector.tensor_scalar_add(gib[:, g:g + 1], in0=pg[:, :],
                                        scalar1=dc[:, 1 + g:2 + g])
        h1 = sb.tile([PH, 1], F32)
        nc.vector.tensor_copy(h1[:, :], h0[:, 0:1])
        h2 = sb.tile([PH, 1], F32)
        nc.vector.tensor_copy(h2[:, :], h0[:, 1:2])

        for step in range(L):
            p_r1 = ps.tile([PH, 1], F32, tag="pp")
            p_z1 = ps.tile([PH, 1], F32, tag="pp")
            p_n1 = ps.tile([PH, 1], F32, tag="pp")
            nc.tensor.matmul(p_r1[:, :], whh1[:, 0:48], h1[:, :], start=True, stop=True)
            nc.tensor.matmul(p_z1[:, :], whh1[:, 48:96], h1[:, :], start=True, stop=True)
            nc.tensor.matmul(p_n1[:, :], whh1[:, 96:144], h1[:, :], start=True, stop=True)
            r1 = st.tile([PH, 1], F32, tag="r1")
            nc.scalar.activation(r1[:, :], p_r1[:, :], AF.Sigmoid,
                                 bias=gib[:, 0:1], scale=1.0)
            z1 = st.tile([PH, 1], F32, tag="z1")
            nc.scalar.activation(z1[:, :], p_z1[:, :], AF.Sigmoid,
                                 bias=gib[:, 1:2], scale=1.0)
            t1 = st.tile([PH, 1], F32, tag="t1")
            nc.vector.scalar_tensor_tensor(t1[:, :], p_n1[:, :], dc[:, 0:1],
                                           r1[:, :], op0=ALU.add, op1=ALU.mult)
            n1 = st.tile([PH, 1], F32, tag="n1")
            nc.scalar.activation(n1[:, :], t1[:, :], AF.Tanh,
                                 bias=gib[:, 2:3], scale=1.0)
            d1 = st.tile([PH, 1], F32, tag="d1")
            nc.vector.tensor_sub(d1[:, :], h1[:, :], n1[:, :])
            h1n = st.tile([PH, 1], F32, tag="h1n")
            nc.vector.scalar_tensor_tensor(h1n[:, :], d1[:, :], z1[:, 0:1],
                                           n1[:, :], op0=ALU.mult, op1=ALU.add)
            h1 = h1n

            p_r2 = ps.tile([PH, 1], F32, tag="pp")
            p_z2 = ps.tile([PH, 1], F32, tag="pp")
            p_gn2 = ps.tile([PH, 1], F32, tag="pp")
            p_hn2 = ps.tile([PH, 1], F32, tag="pp")
            nc.tensor.matmul(p_r2[:, :], wih2[:, 0:48], h1[:, :], start=True, stop=False)
            nc.tensor.matmul(p_r2[:, :], whh2[:, 0:48], h2[:, :], start=False, stop=True)
            nc.tensor.matmul(p_z2[:, :], wih2[:, 48:96], h1[:, :], start=True, stop=False)
            nc.tensor.matmul(p_z2[:, :], whh2[:, 48:96], h2[:, :], start=False, stop=True)
            nc.tensor.matmul(p_gn2[:, :], wih2[:, 96:144], h1[:, :], start=True, stop=True)
            nc.tensor.matmul(p_hn2[:, :], whh2[:, 96:144], h2[:, :], start=True, stop=True)
            r2 = st.tile([PH, 1], F32, tag="r2")
            nc.scalar.activation(r2[:, :], p_r2[:, :], AF.Sigmoid,
                                 bias=dc[:, 4:5], scale=1.0)
            z2 = st.tile([PH, 1], F32, tag="z2")
            nc.scalar.activation(z2[:, :], p_z2[:, :], AF.Sigmoid,
                                 bias=dc[:, 5:6], scale=1.0)
            t2 = st.tile([PH, 1], F32, tag="t2")
            nc.vector.scalar_tensor_tensor(t2[:, :], p_hn2[:, :], dc[:, 7:8],
                                           r2[:, :], op0=ALU.add, op1=ALU.mult)
            s2 = st.tile([PH, 1], F32, tag="s2")
            nc.vector.scalar_tensor_tensor(s2[:, :], p_gn2[:, :], dc[:, 6:7],
                                           t2[:, :], op0=ALU.add, op1=ALU.add)
            n2 = st.tile([PH, 1], F32, tag="n2")
            nc.scalar.activation(n2[:, :], s2[:, :], AF.Tanh, bias=0.0, scale=1.0)
            d2 = st.tile([PH, 1], F32, tag="d2")
            nc.vector.tensor_sub(d2[:, :], h2[:, :], n2[:, :])
            h2n = st.tile([PH, 1], F32, tag="h2n")
            nc.vector.scalar_tensor_tensor(h2n[:, :], d2[:, :], z2[:, 0:1],
                                           n2[:, :], op0=ALU.mult, op1=ALU.add)
            h2 = h2n

        # head: o = LN(leaky(h2 @ pre_w.T + pre_b)); o = LN(o @ cls_w.T + cls_b)
        def psum_all(x, nchan, tag):
            p1 = ps.tile([1, 1], F32, tag="pp")
            nc.tensor.matmul(p1[:, :], ones_c[0:nchan, :], x[0:nchan, :],
                             start=True, stop=True)
            s1 = st.tile([1, 1], F32, tag=tag + "s")
            nc.vector.tensor_copy(s1[:, :], p1[:, :])
            p2 = ps.tile([16, 1], F32, tag="pp")
            nc.tensor.matmul(p2[0:nchan, :], ones_r[0:1, 0:nchan], s1[:, :],
                             start=True, stop=True)
            return p2

        def col_ln(x, nchan, w_ap, b_ap):
            red = psum_all(x, nchan, "lnr")
            mu = st.tile([16, 1], F32, tag="lnm")
            nc.scalar.mul(mu[0:nchan, :], red[0:nchan, :], 1.0 / nchan)
            xc = st.tile([16, 1], F32, tag="lnx")
            nc.vector.tensor_sub(xc[0:nchan, :], x[0:nchan, :], mu[0:nchan, :])
            sqc = st.tile([16, 1], F32, tag="lns")
            nc.vector.tensor_mul(sqc[0:nchan, :], xc[0:nchan, :], xc[0:nchan, :])
            sv = psum_all(sqc, nchan, "lnv")
            sdv = st.tile([16, 1], F32, tag="lnd")
            nc.scalar.activation(sdv[0:nchan, :], sv[0:nchan, :], AF.Sqrt,
                                 bias=eps16[0:nchan, 0:1], scale=1.0 / nchan)
            rsv = st.tile([16, 1], F32, tag="lne")
            nc.vector.reciprocal(rsv[0:nchan, :], sdv[0:nchan, :])
            y = st.tile([16, 1], F32, tag="lny")
            nc.vector.tensor_mul(y[0:nchan, :], xc[0:nchan, :], rsv[0:nchan, :])
            nc.vector.tensor_mul(y[0:nchan, :], y[0:nchan, :], w_ap)
            nc.vector.tensor_add(y[0:nchan, :], y[0:nchan, :], b_ap)
            return y

        p_o1 = ps.tile([16, 1], F32, tag="pp")
        nc.tensor.matmul(p_o1[:, :], prew[:, :], h2[:, :], start=True, stop=True)
        o1 = st.tile([16, 1], F32, tag="o1")
        o1b = st.tile([16, 1], F32, tag="o1b")
        nc.vector.tensor_scalar_add(o1b[:, :], in0=p_o1[:, :], scalar1=hc[:, 0:1])
        o1mn = st.tile([16, 1], F32, tag="o1mn")
        nc.vector.tensor_scalar_min(o1mn[:, :], in0=o1b[:, :], scalar1=0.0)
        o1mx = st.tile([16, 1], F32, tag="o1mx")
        nc.vector.tensor_scalar_max(o1mx[:, :], in0=o1b[:, :], scalar1=0.0)
        nc.vector.scalar_tensor_tensor(o1[:, :], o1mn[:, :], 0.01,
                                       o1mx[:, :], op0=ALU.mult, op1=ALU.add)
        y1 = col_ln(o1, 16, hc[0:16, 1:2], hc[0:16, 2:3])
        p_o2 = ps.tile([11, 1], F32, tag="pp")
        nc.tensor.matmul(p_o2[:, :], clsw[:, :], y1[0:16, :], start=True, stop=True)
        o2 = st.tile([16, 1], F32, tag="o2")
        nc.vector.tensor_scalar_add(o2[0:11, :], in0=p_o2[:, :],
                                    scalar1=hc[0:11, 3:4])
        y2 = col_ln(o2, 11, hc[0:11, 4:5], hc[0:11, 5:6])

        nc.gpsimd.dma_start(out=o_out[0, :], in_=y2[0:11, :])
        nc.gpsimd.dma_start(out=hf_out[0, :], in_=h1[:, :])
        nc.gpsimd.dma_start(out=hf_out[1, :], in_=h2[:, :])
        ch.close_all()
    return nc


# ============================ host-side glue ===============================

def im2col_x(xs):
    """xs [N,3,160,96] -> [N,27,162,98]: row 3t+c = channel c shifted so a
    fixed-offset read yields tap t of a pad-1 3x3 conv."""
    n = xs.shape[0]
    out = np.zeros((n, 27, 162, 98), np.float32)
    for t in range(9):
        ti, tj = t // 3, t % 3
        r0, c0 = 2 - ti, 2 - tj
        out[:, 3 * t:3 * t + 3, r0:r0 + 160, c0:c0 + 96] = xs
    return out


def _prep_vgg_weights(vgg_params, norm0_w, norm0_b, cnn_w, cnn_b, norm1_w, norm1_b):
    d = {}
    W0, b0 = vgg_params[0]
    import ml_dtypes
    W0 = np.asarray(W0, np.float32)
    d["w0"] = np.ascontiguousarray(W0.transpose(2, 3, 1, 0).reshape(27, 64))
    for li in range(1, 13):
        W, _ = vgg_params[li]
        W = np.asarray(W, np.float32)
        cin, cout, _, _ = VGG[li]
        cinb, coutb = max(1, cin // 128), max(1, cout // 128)
        cinP, coutP = min(cin, 128), min(cout, 128)
        arr = W.reshape(coutb, coutP, cinb, cinP, 3, 3).transpose(3, 0, 2, 4, 5, 1)
        d[f"w{li}"] = np.ascontiguousarray(arr)
    biases = np.zeros((128, N_BIAS_COL), np.float32)
    for li in range(13):
        b = np.asarray(vgg_params[li][1], np.float32)
        cout = VGG[li][1]
        coutP = min(cout, 128)
        for mb in range(max(1, cout // 128)):
            biases[0:coutP, BIAS_COL[(li, mb)]] = b[mb * 128:mb * 128 + coutP]
    d["biases"] = biases
    d["n0w"] = np.ascontiguousarray(
        np.asarray(norm0_w, np.float32).reshape(4, 128, 15).transpose(1, 0, 2))
    d["n0b"] = np.ascontiguousarray(
        np.asarray(norm0_b, np.float32).reshape(4, 128, 15).transpose(1, 0, 2))
    cw = np.asarray(cnn_w, np.float32)           # [16, 512, 3, 3]
    d["cnnw"] = np.ascontiguousarray(
        cw.reshape(16, 4, 128, 3, 3).transpose(2, 1, 3, 4, 0))
    d["cnnb"] = np.asarray(cnn_b, np.float32).reshape(16, 1)
    d["n1w"] = np.asarray(norm1_w, np.float32).reshape(16, 3)
    d["n1b"] = np.asarray(norm1_b, np.float32).reshape(16, 3)
    return d


def _prep_decode(feats, previous_state, att_w, gru_params, pre_w, pre_b,
                 norm2_w, norm2_b, cls_w, cls_b, norm3_w, norm3_b):
    f = np.asarray(feats, np.float32)
    d = {"f": f, "fT": np.ascontiguousarray(f.T)}
    d["w2col"] = np.ascontiguousarray(
        np.asarray(att_w, np.float32)[0, PH:2 * PH].reshape(PH, 1))
    (wi1, wh1, bi1, bh1), (wi2, wh2, bi2, bh2) = [
        tuple(np.asarray(a, np.float32) for a in g) for g in gru_params]
    d["wih1T"] = np.ascontiguousarray(wi1.T)
    d["whh1T"] = np.ascontiguousarray(wh1.T)
    d["wih2T"] = np.ascontiguousarray(wi2.T)
    d["whh2T"] = np.ascontiguousarray(wh2.T)
    dc = np.zeros((PH, 8), np.float32)
    dc[:, 0] = bh1[96:144]
    dc[:, 1] = bi1[0:48] + bh1[0:48]
    dc[:, 2] = bi1[48:96] + bh1[48:96]
    dc[:, 3] = bi1[96:144]
    dc[:, 4] = bi2[0:48] + bh2[0:48]
    dc[:, 5] = bi2[48:96] + bh2[48:96]
    dc[:, 6] = bi2[96:144]
    dc[:, 7] = bh2[96:144]
    d["dc"] = dc
    d["h0"] = np.ascontiguousarray(
        np.asarray(previous_state, np.float32).reshape(2, PH).T)
    d["prewT"] = np.ascontiguousarray(np.asarray(pre_w, np.float32).T)
    d["clswT"] = np.ascontiguousarray(np.asarray(cls_w, np.float32).T)
    hcol = np.zeros((16, 6), np.float32)
    hcol[0:16, 0] = np.asarray(pre_b, np.float32)
    hcol[0:16, 1] = np.asarray(norm2_w, np.float32)
    hcol[0:16, 2] = np.asarray(norm2_b, np.float32)
    hcol[0:11, 3] = np.asarray(cls_b, np.float32)
    hcol[0:11, 4] = np.asarray(norm3_w, np.float32)
    hcol[0:11, 5] = np.asarray(norm3_b, np.float32)
    d["hc"] = hcol
    return d


_CACHE = {}


def _get_ncs():
    if "vgg" not in _CACHE:
        nc_v = build_vgg_nc()
        fix_multi_waits(nc_v)
        nc_d = build_decode_nc()
        fix_multi_waits(nc_d)
        _CACHE["vgg"] = nc_v
        _CACHE["dec"] = nc_d
    return _CACHE["vgg"], _CACHE["dec"]


def kernel(x, lengths, previous_state, vgg_params, norm0_w, norm0_b, cnn_w, cnn_b,
           norm1_w, norm1_b, att_w, att_b, gru_params, pre_w, pre_b,
           norm2_w, norm2_b, cls_w, cls_b, norm3_w, norm3_b):
    assert int(lengths) == L, f"kernel specialized for lengths={L}"
    x = np.asarray(x, np.float32)
    nc_vgg, nc_dec = _get_ncs()

    import time as _t
    t0 = _t.time()
    wmap = _prep_vgg_weights(vgg_params, norm0_w, norm0_b, cnn_w, cnn_b,
                             norm1_w, norm1_b)
    t1 = _t.time()
    xs = im2col_x(x.reshape(L, 3, 160, 96))
    in_maps = []
    for c in range(NCORES):
        m = dict(wmap)
        m["xs"] = np.ascontiguousarray(xs[c * FPC:(c + 1) * FPC])
        in_maps.append(m)
    t2 = _t.time()
    res = run_bass_kernel_spmd(nc_vgg, in_maps, core_ids=list(range(NCORES)))
    t3 = _t.time()
    feats = np.concatenate([res.results[c]["feats"] for c in range(NCORES)], axis=0)

    dmap = _prep_decode(feats, previous_state, att_w, gru_params, pre_w, pre_b,
                        norm2_w, norm2_b, cls_w, cls_b, norm3_w, norm3_b)
    res2 = run_bass_kernel_spmd(nc_dec, [dmap], core_ids=[0])
    t4 = _t.time()
    import sys as _sys
    print(f"[kernel] wprep {t1-t0:.2f}s im2col {t2-t1:.2f}s vgg {t3-t2:.2f}s "
          f"decode {t4-t3:.2f}s", file=_sys.stderr, flush=True)
    o = np.asarray(res2.results[0]["o"], np.float32).reshape(1, 11)
    hf = np.asarray(res2.results[0]["hf"], np.float32).reshape(2, 1, PH)
    return (o, hf)
